# revision 1
# baseline (speedup 1.0000x reference)
"""ODE-RNN Trainium2 kernel.

Problem: out[b, t*8+i, :] = 2-layer GRU (H=1024) run over the batch dim
(64 steps) of sequence t (30 sequences), with initial hiddens taken from an
RK4-integrated ODE trajectory (8 grid points, shared across all runs).

Strategy (8 NeuronCores, pure data-parallel, no collectives):
  - The ODE trajectory (128 sequential tiny (2,1024) MLP evals, <1.2% of
    FLOPs, latency-serial and weight-streaming-bound on a systolic array)
    is computed on the host in fp32, exactly mirroring the reference math.
  - Core i handles the 30 GRU runs with init traj[i] (data-parallel over the
    240 independent (t,i) runs; weights replicated per core).
  - Per core, the GRU is restructured into 4 phases:
      A: gi1 = X @ wi0.T + bias  (dense, M=2048)             -> DRAM
      B: layer-1 recurrence, 64 steps, state batched M=32    -> h1 states
      C: gi2 = H1states @ wi1.T + bias (dense, M=2048)       -> DRAM
      D: layer-2 recurrence, 64 steps                        -> output
    The recurrent matmuls keep the state transposed ([H,parts] x runs) as the
    PE stationary operand and stream the (resident) recurrent weights as the
    moving operand; the state transpose is maintained with PE transposes.
  - All matmuls run in float32r (1 cycle/row, ~1.5e-4 rel err measured).
"""

import numpy as np

try:
    import concourse.bass as bass  # noqa: F401
except ImportError:  # pragma: no cover - fallback for bare environments
    import sys
    sys.path.insert(0, "/opt/trn_rl_repo")
    import concourse.bass as bass  # noqa: F401

import concourse.mybir as mybir
import concourse.tile as tile
from concourse import bacc
from concourse.bass_utils import run_bass_kernel_spmd
from concourse.masks import make_identity

F32 = mybir.dt.float32
F32R = mybir.dt.float32r
AF = mybir.ActivationFunctionType

H = 1024        # hidden size
G3 = 3 * H      # gate width
KC = H // 128   # K chunks
T = 30          # sequences
R = 32          # padded runs per core (30 real + 2 pad)
NSEG = 8
SUB = 4
NCORES = 8


def build_nc(steps=64):
    """Build the per-core Bass module (same program on all 8 cores)."""
    MT = steps * R            # gi row count (2048 for steps=64)
    MCH = MT // 128           # M chunks (16)
    nc = bacc.Bacc()

    xtr = nc.declare_dram_parameter("xtr", [128, KC, MT], F32R, isOutput=False)
    wi0t = nc.declare_dram_parameter("wi0t", [H, G3], F32R, isOutput=False)
    wh0t = nc.declare_dram_parameter("wh0t", [H, G3], F32R, isOutput=False)
    wi1t = nc.declare_dram_parameter("wi1t", [H, G3], F32R, isOutput=False)
    wh1t = nc.declare_dram_parameter("wh1t", [H, G3], F32R, isOutput=False)
    bias1 = nc.declare_dram_parameter("bias1", [G3], F32, isOutput=False)
    bias2 = nc.declare_dram_parameter("bias2", [G3], F32, isOutput=False)
    bhn1 = nc.declare_dram_parameter("bhn1", [H], F32, isOutput=False)
    bhn2 = nc.declare_dram_parameter("bhn2", [H], F32, isOutput=False)
    h1f0 = nc.declare_dram_parameter("h1f0", [R, H], F32, isOutput=False)
    h2f0 = nc.declare_dram_parameter("h2f0", [R, H], F32, isOutput=False)
    h1t0 = nc.declare_dram_parameter("h1t0", [128, KC, R], F32R, isOutput=False)
    h2t0 = nc.declare_dram_parameter("h2t0", [128, KC, R], F32R, isOutput=False)
    out = nc.declare_dram_parameter("out", [steps, R, H], F32, isOutput=True)

    gi1b = nc.dram_tensor("gi1b", [MT, G3], F32R)
    gi2b = nc.dram_tensor("gi2b", [MT, G3], F32R)
    h1ts = nc.dram_tensor("h1ts", [128, KC, steps, R], F32R)

    def bcast(ap, parts=128):
        return bass.AP(tensor=ap.tensor, offset=ap.offset,
                       ap=[[0, parts]] + list(ap.ap))

    with tile.TileContext(nc) as tc:
        with (
            tc.tile_pool(name="wp", bufs=KC) as wp,
            tc.tile_pool(name="const", bufs=1) as const,
        ):
            # --- constants ---
            bias1_bc = const.tile([128, G3], F32)
            nc.sync.dma_start(out=bias1_bc, in_=bcast(bias1[:]))
            bias2_bc = const.tile([128, G3], F32)
            nc.sync.dma_start(out=bias2_bc, in_=bcast(bias2[:]))
            bhn1_bc = const.tile([R, H], F32)
            nc.sync.dma_start(out=bhn1_bc, in_=bcast(bhn1[:], parts=R))
            bhn2_bc = const.tile([R, H], F32)
            nc.sync.dma_start(out=bhn2_bc, in_=bcast(bhn2[:], parts=R))
            ident_g = const.tile([32, 32], F32)
            make_identity(nc, ident_g)
            ident = const.tile([32, 32], F32)
            nc.vector.tensor_copy(ident, ident_g)
            ident_r = const.tile([32, 32], F32R)
            nc.vector.tensor_copy(ident_r, ident_g)

            # --- state tiles ---
            h1f = const.tile([R, H], F32)
            nc.sync.dma_start(out=h1f, in_=h1f0[:])
            h2f = const.tile([R, H], F32)
            nc.sync.dma_start(out=h2f, in_=h2f0[:])
            h1t = const.tile([128, KC, R], F32R)
            nc.sync.dma_start(out=h1t, in_=h1t0[:])
            h2t = const.tile([128, KC, R], F32R)
            nc.sync.dma_start(out=h2t, in_=h2t0[:])

            def load_weight(param, label):
                tiles = []
                for k in range(KC):
                    wt = wp.tile([128, G3], F32R, tag="w", name=f"w_{label}_{k}")
                    nc.sync.dma_start(out=wt, in_=param[k * 128:(k + 1) * 128, :])
                    tiles.append(wt)
                return tiles

            def phase_gi(wtiles, bias_bc, gib, lhs_loader, nm):
                """gi = lhsT.T @ W + bias for MCH M-chunks of 128 rows."""
                with (
                    tc.tile_pool(name=f"psA{nm}", bufs=2, space="PSUM") as psA,
                    tc.tile_pool(name=f"evp{nm}", bufs=2) as evp,
                    tc.tile_pool(name=f"lhsp{nm}", bufs=2) as lhsp,
                ):
                    lhs = None
                    for m in range(MCH):
                        lhs, msub = lhs_loader(lhsp, m, lhs)
                        for half in range(2):
                            ps = psA.tile([128, 1536], F32, tag="ps", name=f"ps{nm}_{m}_{half}")
                            for n3 in range(3):
                                ncol = half * 1536 + n3 * 512
                                for k in range(KC):
                                    nc.tensor.matmul(
                                        ps[:, n3 * 512:(n3 + 1) * 512],
                                        lhs[:, k, msub * 128:(msub + 1) * 128],
                                        wtiles[k][:, ncol:ncol + 512],
                                        start=(k == 0), stop=(k == KC - 1))
                            ev = evp.tile([128, 1536], F32R, tag="ev", name=f"ev{nm}_{m}_{half}")
                            nc.vector.tensor_add(
                                ev, ps, bias_bc[:, half * 1536:(half + 1) * 1536])
                            nc.sync.dma_start(
                                out=gib[m * 128:(m + 1) * 128,
                                        half * 1536:(half + 1) * 1536],
                                in_=ev)

            def lhs_loader_A(lhsp, m, lhs):
                # xtr chunks: up to 4 M-chunks per DMA ([128, KC, <=512] tiles)
                if m % 4 == 0:
                    width = min(512, (MCH - m) * 128)
                    lhs = lhsp.tile([128, KC, width], F32R, tag="lhsA", name=f"lhsA_{m}")
                    nc.sync.dma_start(
                        out=lhs, in_=xtr[:, :, m * 128:m * 128 + width])
                return lhs, m % 4

            def lhs_loader_C(lhsp, m, lhs):
                # h1ts slice: steps 4m..4m+4 -> [128, KC, 128] (s-major, t-minor)
                lhs = lhsp.tile([128, KC, 4, R], F32R, tag="lhsC", name=f"lhsC_{m}")
                nc.sync.dma_start(out=lhs, in_=h1ts[:, :, 4 * m:4 * m + 4, :])
                return lhs.rearrange("p k s t -> p k (s t)"), 0

            def phase_rec(wtiles, gib, bhn_bc, hf, ht, save, outd, nm):
                with (
                    tc.tile_pool(name=f"ghp{nm}", bufs=7, space="PSUM") as ghp,
                    tc.tile_pool(name=f"trp{nm}", bufs=1, space="PSUM") as trpp,
                    tc.tile_pool(name=f"gp{nm}", bufs=14) as gp,
                    tc.tile_pool(name=f"gip{nm}", bufs=2) as gip,
                ):
                    for s in range(steps):
                        gi = gip.tile([R, G3], F32R, tag="gi", name=f"gi{nm}_{s}")
                        nc.sync.dma_start(out=gi, in_=gib[s * R:(s + 1) * R, :])
                        # K-split accumulation: the k<4 half depends only on
                        # ht chunks 0-3 (rewritten by the previous step's
                        # first gate slice), so it can overlap the previous
                        # step's second-slice gates instead of waiting for
                        # the full state update.
                        ghs = {}
                        for kh in range(2):
                            for n in (0, 2, 4, 1, 3, 5):
                                if kh == 0:
                                    ghs[n] = ghp.tile([R, 512], F32, tag="gh",
                                                      name=f"gh{nm}_{s}_{n}")
                                gh = ghs[n]
                                for k in range(kh * 4, kh * 4 + 4):
                                    nc.tensor.matmul(
                                        gh, ht[:, k, :],
                                        wtiles[k][:, n * 512:(n + 1) * 512],
                                        start=(k == 0),
                                        stop=(k == KC - 1 and n >= 4))
                                if kh == 1 and n < 4:
                                    # r/z gates: accumulate gi (incl. biases)
                                    # on the PE so ACT can sigmoid PSUM
                                    # directly (saves 2 DVE adds per slice).
                                    nc.tensor.matmul(
                                        gh, ident_r, gi[:, n * 512:(n + 1) * 512],
                                        start=False, stop=True)
                        trp = trpp.tile([128, KC, R], F32, tag="tr", name=f"tr{nm}_{s}")
                        for j in range(2):
                            c0 = j * 512
                            t = lambda nmm: gp.tile([R, 512], F32, tag="gt", name=f"{nmm}{nm}_{s}_{j}")
                            rr = t("rr")
                            nc.scalar.activation(rr, ghs[j], AF.Sigmoid)
                            zz = t("zz")
                            nc.scalar.activation(zz, ghs[2 + j], AF.Sigmoid)
                            hn = t("hn")
                            nc.vector.tensor_add(hn, ghs[4 + j], bhn_bc[:, c0:c0 + 512])
                            t1 = t("t1")
                            nc.vector.tensor_mul(t1, rr, hn)
                            npre = t("npre")
                            nc.vector.tensor_add(npre, t1, gi[:, 2 * H + c0:2 * H + c0 + 512])
                            nn = t("nn")
                            nc.scalar.activation(nn, npre, AF.Tanh)
                            dd = t("dd")
                            nc.vector.tensor_sub(dd, hf[:, c0:c0 + 512], nn)
                            t2 = t("t2")
                            nc.vector.tensor_mul(t2, zz, dd)
                            nc.vector.tensor_add(hf[:, c0:c0 + 512], nn, t2)
                            for c in range(j * 4, j * 4 + 4):
                                nc.tensor.transpose(
                                    trp[:, c, :], hf[:, c * 128:(c + 1) * 128], ident)
                            for c in range(j * 4, j * 4 + 4):
                                nc.vector.tensor_copy(ht[:, c, :], trp[:, c, :])
                        if save is not None:
                            nc.sync.dma_start(out=save[:, :, s, :], in_=ht)
                        if outd is not None:
                            nc.sync.dma_start(out=outd[s], in_=hf)

            w = load_weight(wi0t, "i0")
            phase_gi(w, bias1_bc, gi1b, lhs_loader_A, "A")
            w = load_weight(wh0t, "h0")
            phase_rec(w, gi1b, bhn1_bc, h1f, h1t, h1ts, None, "B")
            w = load_weight(wi1t, "i1")
            phase_gi(w, bias2_bc, gi2b, lhs_loader_C, "C")
            w = load_weight(wh1t, "h1")
            phase_rec(w, gi2b, bhn2_bc, h2f, h2t, None, out, "D")

    nc.finalize()
    return nc


def ode_traj(w1, b1, w2, b2, w3, b3):
    """RK4 trajectory of the ODE, mirroring the reference exactly (fp32)."""
    w1t = w1.T.astype(np.float32)
    w2t = w2.T.astype(np.float32)
    w3t = w3.T.astype(np.float32)

    def f(h):
        a = np.tanh(h @ w1t + b1)
        a = np.tanh(a @ w2t + b2)
        return a @ w3t + b3

    dt = np.float32((1.0 / NSEG) / SUB)
    h = np.zeros((2, H), np.float32)
    traj = []
    for _ in range(NSEG):
        for _ in range(SUB):
            k1 = f(h)
            k2 = f(h + np.float32(0.5) * dt * k1)
            k3 = f(h + np.float32(0.5) * dt * k2)
            k4 = f(h + dt * k3)
            h = h + (dt / np.float32(6.0)) * (k1 + np.float32(2.0) * k2
                                              + np.float32(2.0) * k3 + k4)
        traj.append(h.copy())
    return np.stack(traj)  # (NSEG, 2, H)


def make_in_maps(x, w1, b1, w2, b2, w3, b3, wi0, wh0, bi0, bh0,
                 wi1, wh1, bi1, bh1, steps=64, cores=NCORES):
    traj = ode_traj(w1, b1, w2, b2, w3, b3)
    MT = steps * R

    # xtr[p, k, s*R + t] = x[s, t, k*128+p]
    xp = np.zeros((steps, R, H), np.float32)
    xp[:, :T, :] = x[:steps, :, :]
    xtr = np.ascontiguousarray(
        xp.reshape(MT, KC, 128).transpose(2, 1, 0))

    shared = {
        "xtr": xtr,
        "wi0t": np.ascontiguousarray(wi0.T),
        "wh0t": np.ascontiguousarray(wh0.T),
        "wi1t": np.ascontiguousarray(wi1.T),
        "wh1t": np.ascontiguousarray(wh1.T),
        "bias1": np.concatenate([bi0[:2 * H] + bh0[:2 * H], bi0[2 * H:]]),
        "bias2": np.concatenate([bi1[:2 * H] + bh1[:2 * H], bi1[2 * H:]]),
        "bhn1": np.ascontiguousarray(bh0[2 * H:]),
        "bhn2": np.ascontiguousarray(bh1[2 * H:]),
    }
    in_maps = []
    for i in range(cores):
        h1 = traj[i, 0]
        h2 = traj[i, 1]
        m = dict(shared)
        m["h1f0"] = np.ascontiguousarray(np.tile(h1, (R, 1)))
        m["h2f0"] = np.ascontiguousarray(np.tile(h2, (R, 1)))
        m["h1t0"] = np.ascontiguousarray(
            np.broadcast_to(h1.reshape(KC, 128).T[:, :, None], (128, KC, R)))
        m["h2t0"] = np.ascontiguousarray(
            np.broadcast_to(h2.reshape(KC, 128).T[:, :, None], (128, KC, R)))
        in_maps.append(m)
    return in_maps


_NC_CACHE = {}


def _get_nc(steps):
    if steps not in _NC_CACHE:
        _NC_CACHE[steps] = build_nc(steps)
    return _NC_CACHE[steps]


def run_cores(inputs, steps=64, cores=NCORES, **run_kwargs):
    in_maps = make_in_maps(steps=steps, cores=cores, **inputs)
    nc = _get_nc(steps)
    return run_bass_kernel_spmd(nc, in_maps, core_ids=list(range(cores)),
                                **run_kwargs)


def kernel(x, w1, b1, w2, b2, w3, b3, wi0, wh0, bi0, bh0,
           wi1, wh1, bi1, bh1):
    x = np.asarray(x, np.float32)
    args = dict(x=x, w1=w1, b1=b1, w2=w2, b2=b2, w3=w3, b3=b3,
                wi0=wi0, wh0=wh0, bi0=bi0, bh0=bh0,
                wi1=wi1, wh1=wh1, bi1=bi1, bh1=bh1)
    args = {k: np.asarray(v, np.float32) for k, v in args.items()}
    res = run_cores(args, steps=64, cores=NCORES)
    B = 64
    full = np.empty((B, T * NCORES, H), np.float32)
    for i in range(NCORES):
        full[:, i::NCORES, :] = res.results[i]["out"][:, :T, :]
    return full



# revision 7
# speedup vs baseline: 2.4031x; 2.4031x over previous
"""ODE-RNN Trainium2 kernel, v2 (gates-major fused recurrence).

out[b, t*8+i, :] = 2-layer GRU (H=1024) over the batch dim (64 steps) of
sequence t (30 sequences), init hiddens from an RK4 ODE trajectory
(8 grid points).  Core i handles the 30 runs with init traj[i].

Per-core structure:
  Phase A: gi1 = x @ wi0.T + bias, gates-major in f32r, written to DRAM
           as [128, 64 steps, 720]; free cols = (tau{r,z}, chunk, run)
           for [0:480], n-gate (chunk, run) for [480:720].
  Loop (64 steps, both layers fused per step): recurrent matmuls are
           gates-major: out tile [128 gates, 30 runs] in PSUM,
           stationary = bf16 weight tile [128 k, 128 gates], moving =
           bf16 state [128 k, 30 runs] (bf16 moving -> 1 cycle/row).
           gi / biases enter PSUM via f32r identity-matmuls (>=256
           wide).  Layer-2's wi1 matmuls accumulate into the same PSUM
           group as wh1 (rz) or a dedicated bank (n), so there is no
           dense gi2 phase and no h1-state saving.  Elementwise is f32
           on ACT/DVE, split into h-chunk halves to pipeline under the
           PE stream.  State kept twice: f32 master + bf16 PE copy.
"""

import numpy as np

try:
    import concourse.bass as bass  # noqa: F401
except ImportError:  # pragma: no cover
    import sys
    sys.path.insert(0, "/opt/trn_rl_repo")
    import concourse.bass as bass  # noqa: F401

import ml_dtypes
import concourse.mybir as mybir
import concourse.tile as tile
from concourse import bacc
from concourse.bass_utils import run_bass_kernel_spmd
from concourse.masks import make_identity

F32 = mybir.dt.float32
F32R = mybir.dt.float32r
BF16 = mybir.dt.bfloat16
AF = mybir.ActivationFunctionType
OP = mybir.AluOpType

H = 1024
KC = 8          # k chunks of 128
NG = 24         # gate tiles (tau*8 + c)
R = 30          # runs per core (exact, no padding)
RP = 32         # padded runs used in phase A only (psum rows >= 256)
S = 64          # steps (batch-as-sequence)
NSEG = 8
SUB = 4
NCORES = 8
T = 30          # sequences


def build_nc(steps=S):
    nc = bacc.Bacc()

    xtr = nc.declare_dram_parameter("xtr", [128, KC, S, RP], F32R, isOutput=False)
    wi0t = nc.declare_dram_parameter("wi0t", [128, KC, 3 * H], F32R, isOutput=False)
    wt0 = nc.declare_dram_parameter("wt0", [128, KC, 3 * H], BF16, isOutput=False)
    wt1 = nc.declare_dram_parameter("wt1", [128, KC, 3 * H], BF16, isOutput=False)
    wt2 = nc.declare_dram_parameter("wt2", [128, KC, 3 * H], BF16, isOutput=False)
    bias1t = nc.declare_dram_parameter("bias1t", [128, NG], F32, isOutput=False)
    b2rz = nc.declare_dram_parameter("b2rz", [128, 480], F32R, isOutput=False)
    bhn1 = nc.declare_dram_parameter("bhn1", [128, 256], F32R, isOutput=False)
    bhn2 = nc.declare_dram_parameter("bhn2", [128, 256], F32R, isOutput=False)
    bi1n = nc.declare_dram_parameter("bi1n", [128, 256], F32R, isOutput=False)
    h1f0 = nc.declare_dram_parameter("h1f0", [128, 240], F32, isOutput=False)
    h2f0 = nc.declare_dram_parameter("h2f0", [128, 240], F32, isOutput=False)
    h1b0 = nc.declare_dram_parameter("h1b0", [128, 240], BF16, isOutput=False)
    h2b0 = nc.declare_dram_parameter("h2b0", [128, 240], BF16, isOutput=False)
    out = nc.declare_dram_parameter("out", [128, S, 240], F32, isOutput=True)

    gi1d = nc.dram_tensor("gi1d", [128, S, 720], F32R)

    with tile.TileContext(nc) as tc:
        with tc.tile_pool(name="wloop", bufs=1) as wlp:
            # prefetch layer-1 recurrent weights during phase A
            w0t = wlp.tile([128, KC, 3 * H], BF16, tag="w0", name="w0t")
            nc.sync.dma_start(out=w0t, in_=wt0[:])
            w0 = [w0t[:, kc] for kc in range(KC)]

            # ============= Phase A: gi1 (f32r, gates-major) =============
            with (
                tc.tile_pool(name="wApool", bufs=1) as wApool,
                tc.tile_pool(name="xw_pool", bufs=2) as xwp,
                tc.tile_pool(name="gat_pool", bufs=2) as gatp,
                tc.tile_pool(name="psA", bufs=4, space="PSUM") as psA,
                tc.tile_pool(name="constA", bufs=1) as constA,
            ):
                bias1_sb = constA.tile([128, NG], F32)
                nc.sync.dma_start(out=bias1_sb, in_=bias1t[:])

                wiAt = wApool.tile([128, KC, 3 * H], F32R, tag="wiA",
                                   name="wiAt")
                nc.sync.dma_start(out=wiAt, in_=wi0t[:])
                wiA = [wiAt[:, kc] for kc in range(KC)]

                for hb in range(8):  # half-blocks of 8 steps
                    xw = xwp.tile([128, KC, 8, RP], F32R, tag="xw",
                                  name=f"xw_{hb}")
                    nc.sync.dma_start(
                        out=xw, in_=xtr[:, :, hb * 8:(hb + 1) * 8, :])
                    gat = gatp.tile([128, 8, 720], F32R, tag="gat",
                                    name=f"gat_{hb}")
                    for g in range(NG):
                        ps = psA.tile([128, 8, RP], F32, tag="psA",
                                      name=f"psA_{hb}_{g}")
                        for kc in range(KC):
                            nc.tensor.matmul(
                                ps, wiA[kc][:, g * 128:(g + 1) * 128],
                                xw[:, kc],
                                start=(kc == 0), stop=(kc == KC - 1))
                        tau, c = g // 8, g % 8
                        off = tau * 240 + c * 30 if tau < 2 else 480 + c * 30
                        nc.vector.tensor_scalar_add(
                            gat[:, :, off:off + 30], ps[:, :, 0:30],
                            bias1_sb[:, g:g + 1])
                    nc.sync.dma_start(
                        out=gi1d[:, hb * 8:(hb + 1) * 8, :], in_=gat)

            # ================= Fused recurrence loop ====================
            with (
                tc.tile_pool(name="wloop2", bufs=1) as wlp2,
                tc.tile_pool(name="constL", bufs=1) as constL,
                tc.tile_pool(name="gi_pool", bufs=2) as gip,
                tc.tile_pool(name="st_pool", bufs=2) as stp,
                tc.tile_pool(name="ew_pool", bufs=2) as ewp,
                tc.tile_pool(name="psL", bufs=2, space="PSUM") as psL,
            ):
                w2t = wlp2.tile([128, KC, 3 * H], BF16, tag="w2", name="w2t")
                nc.sync.dma_start(out=w2t, in_=wt2[:])
                w2 = [w2t[:, kc] for kc in range(KC)]
                w1t = wlp2.tile([128, KC, 3 * H], BF16, tag="w1", name="w1t")
                nc.sync.dma_start(out=w1t, in_=wt1[:])
                w1 = [w1t[:, kc] for kc in range(KC)]

                identf = constL.tile([128, 128], F32)
                make_identity(nc, identf)
                identr = constL.tile([128, 128], F32R)
                nc.vector.tensor_copy(identr, identf)

                b2rz_sb = constL.tile([128, 480], F32R)
                nc.sync.dma_start(out=b2rz_sb, in_=b2rz[:])
                bhn1_sb = constL.tile([128, 256], F32R)
                nc.sync.dma_start(out=bhn1_sb, in_=bhn1[:])
                bhn2_sb = constL.tile([128, 256], F32R)
                nc.sync.dma_start(out=bhn2_sb, in_=bhn2[:])
                bi1n_sb = constL.tile([128, 256], F32R)
                nc.sync.dma_start(out=bi1n_sb, in_=bi1n[:])

                h1f = stp.tile([128, 240], F32, tag="h1f", name="h1f_init")
                nc.sync.dma_start(out=h1f, in_=h1f0[:])
                h2f = stp.tile([128, 240], F32, tag="h2f", name="h2f_init")
                nc.sync.dma_start(out=h2f, in_=h2f0[:])
                h1b = stp.tile([128, 240], BF16, tag="h1b", name="h1b_init")
                nc.sync.dma_start(out=h1b, in_=h1b0[:])
                h2b = stp.tile([128, 240], BF16, tag="h2b", name="h2b_init")
                nc.sync.dma_start(out=h2b, in_=h2b0[:])

                def load_gi(b):
                    t = gip.tile([128, 2, 720], F32R, tag="gw", name=f"gw_{b}")
                    nc.sync.dma_start(out=t, in_=gi1d[:, b * 2:(b + 1) * 2, :])
                    return t

                gtiles = [load_gi(0), load_gi(1)]

                def rec_mms(dst_rz, dst_n, wts, mov, kcs, stop_rz, stop_n):
                    """Gate matmuls for one layer pass: rz slices into
                    dst_rz (480 wide), n slices into dst_n (240 wide)."""
                    last = kcs[-1]
                    for c in range(8):
                        for tau in range(3):
                            g = tau * 8 + c
                            if tau < 2:
                                dst = dst_rz[:, tau * 240 + c * 30:
                                             tau * 240 + c * 30 + 30]
                                stop_k = last if stop_rz else -1
                            else:
                                dst = dst_n[:, c * 30:c * 30 + 30]
                                stop_k = last if stop_n else -1
                            for kc in kcs:
                                nc.tensor.matmul(
                                    dst,
                                    wts[kc][:, g * 128:(g + 1) * 128],
                                    mov[:, kc * 30:(kc + 1) * 30],
                                    start=False,
                                    stop=(kc == stop_k))

                def elementwise(lab, s, hf, Trz, Tn, hfp, ginA, hf_new, hb_new):
                    """GRU combine for h-chunk half hf (cols hf*120..+120)."""
                    lo = hf * 120
                    t = lambda nm: ewp.tile(
                        [128, 120], F32, tag=f"{nm}{hf}{lab}",
                        name=f"{nm}_{lab}_{s}_{hf}")
                    rz = ewp.tile([128, 2, 120], F32, tag=f"rz{hf}{lab}",
                                  name=f"rz_{lab}_{s}_{hf}")
                    nc.scalar.activation(
                        rz,
                        Trz.rearrange("p (t x) -> p t x", t=2)[:, :, lo:lo + 120],
                        AF.Sigmoid)
                    oz = t("oz")
                    nc.vector.tensor_scalar(oz, rz[:, 1], -1.0, 1.0,
                                            OP.mult, OP.add)
                    bz = t("bz")
                    nc.vector.tensor_mul(bz, rz[:, 1], hfp[:, lo:lo + 120])
                    t1 = t("t1")
                    nc.vector.tensor_mul(t1, rz[:, 0], Tn[:, lo:lo + 120])
                    npre = t("np")
                    nc.vector.tensor_add(npre, t1, ginA)
                    nn = t("nn")
                    nc.scalar.activation(nn, npre, AF.Tanh)
                    aa = t("aa")
                    nc.vector.tensor_mul(aa, nn, oz)
                    nc.vector.tensor_add(hf_new[:, lo:lo + 120], aa, bz)
                    nc.vector.tensor_add(hb_new[:, lo:lo + 120], aa, bz)

                for s in range(steps):
                    b, j = divmod(s, 2)
                    T1 = psL.tile([128, 480], F32, tag="T1", name=f"T1_{s}")
                    T2 = psL.tile([128, 512], F32, tag="T2", name=f"T2_{s}")
                    T3 = psL.tile([128, 480], F32, tag="T3", name=f"T3_{s}")
                    T4 = psL.tile([128, 256], F32, tag="T4", name=f"T4_{s}")
                    g = gtiles[b]

                    # L1 psum init + recurrent matmuls
                    nc.tensor.matmul(T1, identr, g[:, j, 0:480],
                                     start=True, stop=False)
                    nc.tensor.matmul(T2[:, 0:256], identr, bhn1_sb,
                                     start=True, stop=False)
                    rec_mms(T1, T2[:, 0:240], w0, h1b, list(range(KC)),
                            True, True)

                    # L2 psum init + wh1 matmuls (independent of E1)
                    nc.tensor.matmul(T3, identr, b2rz_sb, start=True, stop=False)
                    nc.tensor.matmul(T2[:, 256:512], identr, bhn2_sb,
                                     start=True, stop=False)
                    nc.tensor.matmul(T4, identr, bi1n_sb, start=True, stop=False)
                    rec_mms(T3, T2[:, 256:496], w2, h2b, list(range(KC)),
                            False, True)

                    # E1: layer-1 gate combine, by halves
                    h1f_new = stp.tile([128, 240], F32, tag="h1f",
                                       name=f"h1f_{s}")
                    h1b_new = stp.tile([128, 240], BF16, tag="h1b",
                                       name=f"h1b_{s}")
                    for hf in range(2):
                        elementwise(
                            "a", s, hf, T1, T2[:, 0:240], h1f,
                            g[:, j, 480 + hf * 120:480 + hf * 120 + 120],
                            h1f_new, h1b_new)

                    # prefetch next gi window (framework delays the DMA
                    # until the 2-ago window's readers are done)
                    if j == 0 and b + 2 < steps // 2:
                        gtiles.append(load_gi(b + 2))

                    # L2 wi1 matmuls, k-halves so they chase E1 halves
                    rec_mms(T3, T4[:, 0:240], w1, h1b_new, [0, 1, 2, 3],
                            False, False)
                    rec_mms(T3, T4[:, 0:240], w1, h1b_new, [4, 5, 6, 7],
                            True, True)

                    # E2: layer-2 gate combine
                    h2f_new = stp.tile([128, 240], F32, tag="h2f",
                                       name=f"h2f_{s}")
                    h2b_new = stp.tile([128, 240], BF16, tag="h2b",
                                       name=f"h2b_{s}")
                    for hf in range(2):
                        elementwise(
                            "b", s, hf, T3, T2[:, 256:496], h2f,
                            T4[:, hf * 120:hf * 120 + 120],
                            h2f_new, h2b_new)

                    nc.sync.dma_start(out=out[:, s, :], in_=h2f_new)

                    h1f, h1b, h2f, h2b = h1f_new, h1b_new, h2f_new, h2b_new

    nc.finalize()
    return nc


def ode_traj(w1, b1, w2, b2, w3, b3):
    """RK4 trajectory of the ODE, mirroring the reference exactly (fp32)."""
    w1t = w1.T.astype(np.float32)
    w2t = w2.T.astype(np.float32)
    w3t = w3.T.astype(np.float32)

    def f(h):
        a = np.tanh(h @ w1t + b1)
        a = np.tanh(a @ w2t + b2)
        return a @ w3t + b3

    dt = np.float32((1.0 / NSEG) / SUB)
    h = np.zeros((2, H), np.float32)
    traj = []
    for _ in range(NSEG):
        for _ in range(SUB):
            k1 = f(h)
            k2 = f(h + np.float32(0.5) * dt * k1)
            k3 = f(h + np.float32(0.5) * dt * k2)
            k4 = f(h + dt * k3)
            h = h + (dt / np.float32(6.0)) * (k1 + np.float32(2.0) * k2
                                              + np.float32(2.0) * k3 + k4)
        traj.append(h.copy())
    return np.stack(traj)  # (NSEG, 2, H)


def _bc_runs(per_gate, width):
    """[G] gate-vector -> [128, width] broadcast over 30 runs; G = n*128,
    cols laid out (chunk, run) with zero padding to `width`."""
    nchunk = per_gate.size // 128
    a = per_gate.reshape(nchunk, 128)  # [chunk, p]
    o = np.zeros((128, width), np.float32)
    o[:, :nchunk * 30] = np.repeat(
        a.T[:, :, None], 30, axis=2).reshape(128, nchunk * 30)
    return o


def make_in_maps(x, w1, b1, w2, b2, w3, b3, wi0, wh0, bi0, bh0,
                 wi1, wh1, bi1, bh1, cores=NCORES):
    traj = ode_traj(w1, b1, w2, b2, w3, b3)
    bf = ml_dtypes.bfloat16

    xtr = np.zeros((128, KC, S, RP), np.float32)
    # xtr[p, kc, s, r] = x[s, r, kc*128+p]
    xtr[:, :, :, :T] = np.ascontiguousarray(
        x.reshape(S, T, KC, 128).transpose(3, 2, 0, 1))

    bias1 = np.concatenate([bi0[:2 * H] + bh0[:2 * H], bi0[2 * H:]])

    shared = {
        "xtr": xtr,
        "wi0t": np.ascontiguousarray(
            wi0.T.reshape(KC, 128, 3 * H).transpose(1, 0, 2)),
        "wt0": np.ascontiguousarray(
            wh0.T.reshape(KC, 128, 3 * H).transpose(1, 0, 2)).astype(bf),
        "wt1": np.ascontiguousarray(
            wi1.T.reshape(KC, 128, 3 * H).transpose(1, 0, 2)).astype(bf),
        "wt2": np.ascontiguousarray(
            wh1.T.reshape(KC, 128, 3 * H).transpose(1, 0, 2)).astype(bf),
        "bias1t": np.ascontiguousarray(bias1.reshape(NG, 128).T),
        "b2rz": _bc_runs((bi1 + bh1)[:2 * H], 480),
        "bhn1": _bc_runs(bh0[2 * H:], 256),
        "bhn2": _bc_runs(bh1[2 * H:], 256),
        "bi1n": _bc_runs(bi1[2 * H:], 256),
    }
    in_maps = []
    for i in range(cores):
        m = dict(shared)
        for li, nm in ((0, "h1"), (1, "h2")):
            hf = np.repeat(traj[i, li].reshape(KC, 128).T[:, :, None],
                           30, axis=2).reshape(128, 240)
            m[f"{nm}f0"] = np.ascontiguousarray(hf)
            m[f"{nm}b0"] = np.ascontiguousarray(hf).astype(bf)
        in_maps.append(m)
    return in_maps


_NC_CACHE = {}


def _get_nc(steps=S):
    if steps not in _NC_CACHE:
        _NC_CACHE[steps] = build_nc(steps)
    return _NC_CACHE[steps]


def run_cores(inputs, steps=S, cores=NCORES, **run_kwargs):
    in_maps = make_in_maps(cores=cores, **inputs)
    nc = _get_nc(steps)
    return run_bass_kernel_spmd(nc, in_maps, core_ids=list(range(cores)),
                                **run_kwargs)


def kernel(x, w1, b1, w2, b2, w3, b3, wi0, wh0, bi0, bh0,
           wi1, wh1, bi1, bh1):
    args = dict(x=x, w1=w1, b1=b1, w2=w2, b2=b2, w3=w3, b3=b3,
                wi0=wi0, wh0=wh0, bi0=bi0, bh0=bh0,
                wi1=wi1, wh1=wh1, bi1=bi1, bh1=bh1)
    args = {k: np.asarray(v, np.float32) for k, v in args.items()}
    res = run_cores(args, steps=S, cores=NCORES)
    B = 64
    full = np.empty((B, T * NCORES, H), np.float32)
    for i in range(NCORES):
        o = np.asarray(res.results[i]["out"], np.float32)
        # out[p, s, c*30+t] -> full[s, t*8+i, c*128+p]
        full[:, i::NCORES, :] = o.reshape(
            128, S, KC, 30).transpose(1, 3, 2, 0).reshape(B, T, H)
    return full


# revision 9
# speedup vs baseline: 2.9965x; 1.2469x over previous
"""ODE-RNN Trainium2 kernel, v2 (gates-major fused recurrence).

out[b, t*8+i, :] = 2-layer GRU (H=1024) over the batch dim (64 steps) of
sequence t (30 sequences), init hiddens from an RK4 ODE trajectory
(8 grid points).  Core i handles the 30 runs with init traj[i].

Per-core structure:
  Phase A: gi1 = x @ wi0.T + bias, gates-major in f32r, written to DRAM
           as [128, 64 steps, 720]; free cols = (tau{r,z}, chunk, run)
           for [0:480], n-gate (chunk, run) for [480:720].
  Loop (64 steps, both layers fused per step): recurrent matmuls are
           gates-major: out tile [128 gates, 30 runs] in PSUM,
           stationary = bf16 weight tile [128 k, 128 gates], moving =
           bf16 state [128 k, 30 runs] (bf16 moving -> 1 cycle/row).
           gi / biases enter PSUM via f32r identity-matmuls (>=256
           wide).  Layer-2's wi1 matmuls accumulate into the same PSUM
           group as wh1 (rz) or a dedicated bank (n), so there is no
           dense gi2 phase and no h1-state saving.  Elementwise is f32
           on ACT/DVE, split into h-chunk halves to pipeline under the
           PE stream.  State kept twice: f32 master + bf16 PE copy.
"""

import numpy as np

try:
    import concourse.bass as bass  # noqa: F401
except ImportError:  # pragma: no cover
    import sys
    sys.path.insert(0, "/opt/trn_rl_repo")
    import concourse.bass as bass  # noqa: F401

import ml_dtypes
import concourse.mybir as mybir
import concourse.tile as tile
from concourse import bacc
from concourse.bass_utils import run_bass_kernel_spmd
from concourse.masks import make_identity

F32 = mybir.dt.float32
F32R = mybir.dt.float32r
BF16 = mybir.dt.bfloat16
AF = mybir.ActivationFunctionType
OP = mybir.AluOpType

H = 1024
KC = 8          # k chunks of 128
NG = 24         # gate tiles (tau*8 + c)
R = 30          # runs per core (exact, no padding)
RP = 32         # padded runs used in phase A only (psum rows >= 256)
S = 64          # steps (batch-as-sequence)
NSEG = 8
SUB = 4
NCORES = 8
T = 30          # sequences


def build_nc(steps=S):
    nc = bacc.Bacc()

    xtr = nc.declare_dram_parameter("xtr", [128, KC, S, RP], F32R, isOutput=False)
    wi0t = nc.declare_dram_parameter("wi0t", [128, KC, 3 * H], F32R, isOutput=False)
    wt0 = nc.declare_dram_parameter("wt0", [128, KC, 3 * H], BF16, isOutput=False)
    wt1 = nc.declare_dram_parameter("wt1", [128, KC, 3 * H], BF16, isOutput=False)
    wt2 = nc.declare_dram_parameter("wt2", [128, KC, 3 * H], BF16, isOutput=False)
    bias1t = nc.declare_dram_parameter("bias1t", [128, NG], F32, isOutput=False)
    b2rz = nc.declare_dram_parameter("b2rz", [128, 480], F32R, isOutput=False)
    bhn1 = nc.declare_dram_parameter("bhn1", [128, 256], F32R, isOutput=False)
    bhn2 = nc.declare_dram_parameter("bhn2", [128, 256], F32R, isOutput=False)
    bi1n = nc.declare_dram_parameter("bi1n", [128, 256], F32R, isOutput=False)
    h1f0 = nc.declare_dram_parameter("h1f0", [128, 240], F32, isOutput=False)
    h2f0 = nc.declare_dram_parameter("h2f0", [128, 240], F32, isOutput=False)
    h1b0 = nc.declare_dram_parameter("h1b0", [128, 240], BF16, isOutput=False)
    h2b0 = nc.declare_dram_parameter("h2b0", [128, 240], BF16, isOutput=False)
    out = nc.declare_dram_parameter("out", [128, S, 240], F32, isOutput=True)

    gi1d = nc.dram_tensor("gi1d", [128, S, 720], F32R)

    with tile.TileContext(nc) as tc:
        with tc.tile_pool(name="wloop", bufs=1) as wlp:
            # layer-1 recurrent weights, prefetched during phase A
            # (emitted after phase A's own input DMAs; chunked so early
            # consumers unblock as chunks land)
            w0t = wlp.tile([128, KC, 3 * H], BF16, tag="w0", name="w0t")
            w0 = [w0t[:, kc] for kc in range(KC)]

            # ============= Phase A: gi1 (f32r, gates-major) =============
            with (
                tc.tile_pool(name="wApool", bufs=1) as wApool,
                tc.tile_pool(name="xw_pool", bufs=2) as xwp,
                tc.tile_pool(name="gat_pool", bufs=2) as gatp,
                tc.tile_pool(name="psA", bufs=4, space="PSUM") as psA,
                tc.tile_pool(name="constA", bufs=1) as constA,
            ):
                bias1_sb = constA.tile([128, NG], F32)
                nc.sync.dma_start(out=bias1_sb, in_=bias1t[:])

                wiAt = wApool.tile([128, KC, 3 * H], F32R, tag="wiA",
                                   name="wiAt")
                wiA = [wiAt[:, kc] for kc in range(KC)]
                for kc in range(KC):
                    nc.sync.dma_start(out=wiAt[:, kc], in_=wi0t[:, kc])
                for kc in range(KC):
                    nc.sync.dma_start(out=w0t[:, kc], in_=wt0[:, kc])

                for hb in range(8):  # half-blocks of 8 steps
                    xw = xwp.tile([128, KC, 8, RP], F32R, tag="xw",
                                  name=f"xw_{hb}")
                    nc.sync.dma_start(
                        out=xw, in_=xtr[:, :, hb * 8:(hb + 1) * 8, :])
                    gat = gatp.tile([128, 8, 720], F32R, tag="gat",
                                    name=f"gat_{hb}")
                    for g in range(NG):
                        ps = psA.tile([128, 8, RP], F32, tag="psA",
                                      name=f"psA_{hb}_{g}")
                        for kc in range(KC):
                            nc.tensor.matmul(
                                ps, wiA[kc][:, g * 128:(g + 1) * 128],
                                xw[:, kc],
                                start=(kc == 0), stop=(kc == KC - 1))
                        tau, c = g // 8, g % 8
                        off = tau * 240 + c * 30 if tau < 2 else 480 + c * 30
                        nc.vector.tensor_scalar_add(
                            gat[:, :, off:off + 30], ps[:, :, 0:30],
                            bias1_sb[:, g:g + 1])
                    nc.sync.dma_start(
                        out=gi1d[:, hb * 8:(hb + 1) * 8, :], in_=gat)

            # ================= Fused recurrence loop ====================
            with (
                tc.tile_pool(name="wloop2", bufs=1) as wlp2,
                tc.tile_pool(name="constL", bufs=1) as constL,
                tc.tile_pool(name="gi_pool", bufs=2) as gip,
                tc.tile_pool(name="st_pool", bufs=2) as stp,
                tc.tile_pool(name="ew_pool", bufs=2) as ewp,
                tc.tile_pool(name="psL", bufs=2, space="PSUM") as psL,
            ):
                w2t = wlp2.tile([128, KC, 3 * H], BF16, tag="w2", name="w2t")
                w2 = [w2t[:, kc] for kc in range(KC)]
                w1t = wlp2.tile([128, KC, 3 * H], BF16, tag="w1", name="w1t")
                w1 = [w1t[:, kc] for kc in range(KC)]

                identf = constL.tile([128, 128], F32)
                make_identity(nc, identf)
                identr = constL.tile([128, 128], F32R)
                nc.vector.tensor_copy(identr, identf)

                b2rz_sb = constL.tile([128, 480], F32R)
                nc.sync.dma_start(out=b2rz_sb, in_=b2rz[:])
                bhn1_sb = constL.tile([128, 256], F32R)
                nc.sync.dma_start(out=bhn1_sb, in_=bhn1[:])
                bhn2_sb = constL.tile([128, 256], F32R)
                nc.sync.dma_start(out=bhn2_sb, in_=bhn2[:])
                bi1n_sb = constL.tile([128, 256], F32R)
                nc.sync.dma_start(out=bi1n_sb, in_=bi1n[:])

                h1f = stp.tile([128, 240], F32, tag="h1f", name="h1f_init")
                nc.sync.dma_start(out=h1f, in_=h1f0[:])
                h2f = stp.tile([128, 240], F32, tag="h2f", name="h2f_init")
                nc.sync.dma_start(out=h2f, in_=h2f0[:])
                h1b = stp.tile([128, 240], BF16, tag="h1b", name="h1b_init")
                nc.sync.dma_start(out=h1b, in_=h1b0[:])
                h2b = stp.tile([128, 240], BF16, tag="h2b", name="h2b_init")
                nc.sync.dma_start(out=h2b, in_=h2b0[:])

                def load_gi(b):
                    t = gip.tile([128, 2, 720], F32R, tag="gw", name=f"gw_{b}")
                    nc.sync.dma_start(out=t, in_=gi1d[:, b * 2:(b + 1) * 2, :])
                    return t

                gtiles = [load_gi(0), load_gi(1)]

                # weight loads after the loop's warm-up inputs; wi1 before
                # wh1 since the reordered pipeline consumes wi1 first
                for kc in range(KC):
                    nc.sync.dma_start(out=w1t[:, kc], in_=wt1[:, kc])
                for kc in range(KC):
                    nc.sync.dma_start(out=w2t[:, kc], in_=wt2[:, kc])

                def rec_mms(dst_rz, dst_n, wts, mov, kcs, stop_rz, stop_n):
                    """Gate matmuls for one layer pass: rz slices into
                    dst_rz (480 wide), n slices into dst_n (240 wide)."""
                    last = kcs[-1]
                    for c in range(8):
                        for tau in range(3):
                            g = tau * 8 + c
                            if tau < 2:
                                dst = dst_rz[:, tau * 240 + c * 30:
                                             tau * 240 + c * 30 + 30]
                                stop_k = last if stop_rz else -1
                            else:
                                dst = dst_n[:, c * 30:c * 30 + 30]
                                stop_k = last if stop_n else -1
                            for kc in kcs:
                                nc.tensor.matmul(
                                    dst,
                                    wts[kc][:, g * 128:(g + 1) * 128],
                                    mov[:, kc * 30:(kc + 1) * 30],
                                    start=False,
                                    stop=(kc == stop_k))

                def elementwise(lab, s, hf, Trz, Tn, hfp, ginA, hf_new, hb_new):
                    """GRU combine for h-chunk half hf (cols hf*120..+120)."""
                    lo = hf * 120
                    t = lambda nm: ewp.tile(
                        [128, 120], F32, tag=f"{nm}{hf}{lab}",
                        name=f"{nm}_{lab}_{s}_{hf}")
                    rz = ewp.tile([128, 2, 120], F32, tag=f"rz{hf}{lab}",
                                  name=f"rz_{lab}_{s}_{hf}")
                    nc.scalar.activation(
                        rz,
                        Trz.rearrange("p (t x) -> p t x", t=2)[:, :, lo:lo + 120],
                        AF.Sigmoid)
                    oz = t("oz")
                    nc.vector.tensor_scalar(oz, rz[:, 1], -1.0, 1.0,
                                            OP.mult, OP.add)
                    bz = t("bz")
                    nc.vector.tensor_mul(bz, rz[:, 1], hfp[:, lo:lo + 120])
                    t1 = t("t1")
                    nc.vector.tensor_mul(t1, rz[:, 0], Tn[:, lo:lo + 120])
                    npre = t("np")
                    nc.vector.tensor_add(npre, t1, ginA)
                    nn = t("nn")
                    nc.scalar.activation(nn, npre, AF.Tanh)
                    aa = t("aa")
                    nc.vector.tensor_mul(aa, nn, oz)
                    nc.vector.tensor_add(hf_new[:, lo:lo + 120], aa, bz)
                    nc.vector.tensor_add(hb_new[:, lo:lo + 120], aa, bz)

                # Software pipeline: iteration i runs L1 matmuls of
                # step i and L2 matmuls of step i-1, so each elementwise
                # chain has a full matmul section of PE work to hide under.
                T1s, T2s = {}, {}
                h1 = {-1: (h1f, h1b)}
                h2 = {-1: (h2f, h2b)}

                for it in range(steps + 1):
                    sL1, sL2 = it, it - 1

                    if sL1 < steps:
                        b, j = divmod(sL1, 2)
                        T1 = psL.tile([128, 480], F32, tag="T1",
                                      name=f"T1_{sL1}")
                        T2 = psL.tile([128, 512], F32, tag="T2",
                                      name=f"T2_{sL1}")
                        T1s[sL1], T2s[sL1] = T1, T2
                        g = gtiles[b]
                        nc.tensor.matmul(T1, identr, g[:, j, 0:480],
                                         start=True, stop=False)
                        nc.tensor.matmul(T2[:, 0:256], identr, bhn1_sb,
                                         start=True, stop=False)
                        rec_mms(T1, T2[:, 0:240], w0, h1[sL1 - 1][1],
                                list(range(KC)), True, True)

                        # E1(sL1)
                        h1f_new = stp.tile([128, 240], F32, tag="h1f",
                                           name=f"h1f_{sL1}")
                        h1b_new = stp.tile([128, 240], BF16, tag="h1b",
                                           name=f"h1b_{sL1}")
                        for hf in range(2):
                            elementwise(
                                "a", sL1, hf, T1, T2[:, 0:240],
                                h1[sL1 - 1][0],
                                g[:, j, 480 + hf * 120:480 + hf * 120 + 120],
                                h1f_new, h1b_new)
                        h1[sL1] = (h1f_new, h1b_new)

                        if j == 0 and b + 2 < steps // 2:
                            gtiles.append(load_gi(b + 2))

                    if sL2 >= 0:
                        T3 = psL.tile([128, 480], F32, tag="T3",
                                      name=f"T3_{sL2}")
                        T4 = psL.tile([128, 256], F32, tag="T4",
                                      name=f"T4_{sL2}")
                        T2p = T2s.pop(sL2)
                        nc.tensor.matmul(T3, identr, b2rz_sb,
                                         start=True, stop=False)
                        nc.tensor.matmul(T2p[:, 256:512], identr, bhn2_sb,
                                         start=True, stop=False)
                        nc.tensor.matmul(T4, identr, bi1n_sb,
                                         start=True, stop=False)
                        # wi1 first (h1b(sL2) is a full iteration old), then
                        # wh1 (its h2b dep is the freshest elementwise)
                        rec_mms(T3, T4[:, 0:240], w1, h1[sL2][1],
                                list(range(KC)), False, True)
                        rec_mms(T3, T2p[:, 256:496], w2, h2[sL2 - 1][1],
                                list(range(KC)), True, True)

                        # E2(sL2)
                        h2f_new = stp.tile([128, 240], F32, tag="h2f",
                                           name=f"h2f_{sL2}")
                        h2b_new = stp.tile([128, 240], BF16, tag="h2b",
                                           name=f"h2b_{sL2}")
                        for hf in range(2):
                            elementwise(
                                "b", sL2, hf, T3, T2p[:, 256:496],
                                h2[sL2 - 1][0],
                                T4[:, hf * 120:hf * 120 + 120],
                                h2f_new, h2b_new)
                        h2[sL2] = (h2f_new, h2b_new)
                        nc.sync.dma_start(out=out[:, sL2, :], in_=h2f_new)

                        h1.pop(sL2 - 1, None)
                        h2.pop(sL2 - 2, None)
                        T1s.pop(sL2, None)

    nc.finalize()
    return nc


def ode_traj(w1, b1, w2, b2, w3, b3):
    """RK4 trajectory of the ODE, mirroring the reference exactly (fp32)."""
    w1t = w1.T.astype(np.float32)
    w2t = w2.T.astype(np.float32)
    w3t = w3.T.astype(np.float32)

    def f(h):
        a = np.tanh(h @ w1t + b1)
        a = np.tanh(a @ w2t + b2)
        return a @ w3t + b3

    dt = np.float32((1.0 / NSEG) / SUB)
    h = np.zeros((2, H), np.float32)
    traj = []
    for _ in range(NSEG):
        for _ in range(SUB):
            k1 = f(h)
            k2 = f(h + np.float32(0.5) * dt * k1)
            k3 = f(h + np.float32(0.5) * dt * k2)
            k4 = f(h + dt * k3)
            h = h + (dt / np.float32(6.0)) * (k1 + np.float32(2.0) * k2
                                              + np.float32(2.0) * k3 + k4)
        traj.append(h.copy())
    return np.stack(traj)  # (NSEG, 2, H)


def _bc_runs(per_gate, width):
    """[G] gate-vector -> [128, width] broadcast over 30 runs; G = n*128,
    cols laid out (chunk, run) with zero padding to `width`."""
    nchunk = per_gate.size // 128
    a = per_gate.reshape(nchunk, 128)  # [chunk, p]
    o = np.zeros((128, width), np.float32)
    o[:, :nchunk * 30] = np.repeat(
        a.T[:, :, None], 30, axis=2).reshape(128, nchunk * 30)
    return o


def make_in_maps(x, w1, b1, w2, b2, w3, b3, wi0, wh0, bi0, bh0,
                 wi1, wh1, bi1, bh1, cores=NCORES):
    traj = ode_traj(w1, b1, w2, b2, w3, b3)
    bf = ml_dtypes.bfloat16

    xtr = np.zeros((128, KC, S, RP), np.float32)
    # xtr[p, kc, s, r] = x[s, r, kc*128+p]
    xtr[:, :, :, :T] = np.ascontiguousarray(
        x.reshape(S, T, KC, 128).transpose(3, 2, 0, 1))

    bias1 = np.concatenate([bi0[:2 * H] + bh0[:2 * H], bi0[2 * H:]])

    shared = {
        "xtr": xtr,
        "wi0t": np.ascontiguousarray(
            wi0.T.reshape(KC, 128, 3 * H).transpose(1, 0, 2)),
        "wt0": np.ascontiguousarray(
            wh0.T.reshape(KC, 128, 3 * H).transpose(1, 0, 2)).astype(bf),
        "wt1": np.ascontiguousarray(
            wi1.T.reshape(KC, 128, 3 * H).transpose(1, 0, 2)).astype(bf),
        "wt2": np.ascontiguousarray(
            wh1.T.reshape(KC, 128, 3 * H).transpose(1, 0, 2)).astype(bf),
        "bias1t": np.ascontiguousarray(bias1.reshape(NG, 128).T),
        "b2rz": _bc_runs((bi1 + bh1)[:2 * H], 480),
        "bhn1": _bc_runs(bh0[2 * H:], 256),
        "bhn2": _bc_runs(bh1[2 * H:], 256),
        "bi1n": _bc_runs(bi1[2 * H:], 256),
    }
    in_maps = []
    for i in range(cores):
        m = dict(shared)
        for li, nm in ((0, "h1"), (1, "h2")):
            hf = np.repeat(traj[i, li].reshape(KC, 128).T[:, :, None],
                           30, axis=2).reshape(128, 240)
            m[f"{nm}f0"] = np.ascontiguousarray(hf)
            m[f"{nm}b0"] = np.ascontiguousarray(hf).astype(bf)
        in_maps.append(m)
    return in_maps


_NC_CACHE = {}


def _get_nc(steps=S):
    if steps not in _NC_CACHE:
        _NC_CACHE[steps] = build_nc(steps)
    return _NC_CACHE[steps]


def run_cores(inputs, steps=S, cores=NCORES, **run_kwargs):
    in_maps = make_in_maps(cores=cores, **inputs)
    nc = _get_nc(steps)
    return run_bass_kernel_spmd(nc, in_maps, core_ids=list(range(cores)),
                                **run_kwargs)


def kernel(x, w1, b1, w2, b2, w3, b3, wi0, wh0, bi0, bh0,
           wi1, wh1, bi1, bh1):
    args = dict(x=x, w1=w1, b1=b1, w2=w2, b2=b2, w3=w3, b3=b3,
                wi0=wi0, wh0=wh0, bi0=bi0, bh0=bh0,
                wi1=wi1, wh1=wh1, bi1=bi1, bh1=bh1)
    args = {k: np.asarray(v, np.float32) for k, v in args.items()}
    res = run_cores(args, steps=S, cores=NCORES)
    B = 64
    full = np.empty((B, T * NCORES, H), np.float32)
    for i in range(NCORES):
        o = np.asarray(res.results[i]["out"], np.float32)
        # out[p, s, c*30+t] -> full[s, t*8+i, c*128+p]
        full[:, i::NCORES, :] = o.reshape(
            128, S, KC, 30).transpose(1, 3, 2, 0).reshape(B, T, H)
    return full


# revision 10
# speedup vs baseline: 3.1156x; 1.0397x over previous
"""ODE-RNN Trainium2 kernel, v2 (gates-major fused recurrence).

out[b, t*8+i, :] = 2-layer GRU (H=1024) over the batch dim (64 steps) of
sequence t (30 sequences), init hiddens from an RK4 ODE trajectory
(8 grid points).  Core i handles the 30 runs with init traj[i].

Per-core structure:
  Phase A: gi1 = x @ wi0.T + bias, gates-major in f32r, written to DRAM
           as [128, 64 steps, 720]; free cols = (tau{r,z}, chunk, run)
           for [0:480], n-gate (chunk, run) for [480:720].
  Loop (64 steps, both layers fused per step): recurrent matmuls are
           gates-major: out tile [128 gates, 30 runs] in PSUM,
           stationary = bf16 weight tile [128 k, 128 gates], moving =
           bf16 state [128 k, 30 runs] (bf16 moving -> 1 cycle/row).
           gi / biases enter PSUM via f32r identity-matmuls (>=256
           wide).  Layer-2's wi1 matmuls accumulate into the same PSUM
           group as wh1 (rz) or a dedicated bank (n), so there is no
           dense gi2 phase and no h1-state saving.  Elementwise is f32
           on ACT/DVE, split into h-chunk halves to pipeline under the
           PE stream.  State kept twice: f32 master + bf16 PE copy.
"""

import numpy as np

try:
    import concourse.bass as bass  # noqa: F401
except ImportError:  # pragma: no cover
    import sys
    sys.path.insert(0, "/opt/trn_rl_repo")
    import concourse.bass as bass  # noqa: F401

import ml_dtypes
import concourse.mybir as mybir
import concourse.tile as tile
from concourse import bacc
from concourse.bass_utils import run_bass_kernel_spmd
from concourse.masks import make_identity

F32 = mybir.dt.float32
F32R = mybir.dt.float32r
BF16 = mybir.dt.bfloat16
AF = mybir.ActivationFunctionType
OP = mybir.AluOpType

H = 1024
KC = 8          # k chunks of 128
NG = 24         # gate tiles (tau*8 + c)
R = 30          # runs per core (exact, no padding)
RP = 32         # padded runs used in phase A only (psum rows >= 256)
S = 64          # steps (batch-as-sequence)
NSEG = 8
SUB = 4
NCORES = 8
T = 30          # sequences


def build_nc(steps=S):
    nc = bacc.Bacc()

    xtr = nc.declare_dram_parameter("xtr", [128, KC, S, RP], F32R, isOutput=False)
    wi0t = nc.declare_dram_parameter("wi0t", [128, KC, 3 * H], BF16, isOutput=False)
    wt0 = nc.declare_dram_parameter("wt0", [128, KC, 3 * H], BF16, isOutput=False)
    wt1 = nc.declare_dram_parameter("wt1", [128, KC, 3 * H], BF16, isOutput=False)
    wt2 = nc.declare_dram_parameter("wt2", [128, KC, 3 * H], BF16, isOutput=False)
    bias1t = nc.declare_dram_parameter("bias1t", [128, NG], F32, isOutput=False)
    b2rz = nc.declare_dram_parameter("b2rz", [128, 480], F32R, isOutput=False)
    bhn1 = nc.declare_dram_parameter("bhn1", [128, 256], F32R, isOutput=False)
    bhn2 = nc.declare_dram_parameter("bhn2", [128, 256], F32R, isOutput=False)
    bi1n = nc.declare_dram_parameter("bi1n", [128, 256], F32R, isOutput=False)
    h1f0 = nc.declare_dram_parameter("h1f0", [128, 240], F32, isOutput=False)
    h2f0 = nc.declare_dram_parameter("h2f0", [128, 240], F32, isOutput=False)
    h1b0 = nc.declare_dram_parameter("h1b0", [128, 240], BF16, isOutput=False)
    h2b0 = nc.declare_dram_parameter("h2b0", [128, 240], BF16, isOutput=False)
    out = nc.declare_dram_parameter("out", [128, S, 240], F32, isOutput=True)

    gi1d = nc.dram_tensor("gi1d", [128, S, 720], F32R)

    with tile.TileContext(nc) as tc:
        with tc.tile_pool(name="wloop", bufs=1) as wlp:
            # layer-1/2 recurrent weights, prefetched during phase A
            # (emitted after phase A's own input DMAs; chunked so early
            # consumers unblock as chunks land)
            w0t = wlp.tile([128, KC, 3 * H], BF16, tag="w0", name="w0t")
            w0 = [w0t[:, kc] for kc in range(KC)]
            w1t = wlp.tile([128, KC, 3 * H], BF16, tag="w1", name="w1t")
            w1 = [w1t[:, kc] for kc in range(KC)]

            # ============= Phase A: gi1 (f32r, gates-major) =============
            with (
                tc.tile_pool(name="wApool", bufs=1) as wApool,
                tc.tile_pool(name="xw_pool", bufs=2) as xwp,
                tc.tile_pool(name="gat_pool", bufs=2) as gatp,
                tc.tile_pool(name="psA", bufs=4, space="PSUM") as psA,
                tc.tile_pool(name="constA", bufs=1) as constA,
            ):
                bias1_sb = constA.tile([128, NG], F32)
                nc.sync.dma_start(out=bias1_sb, in_=bias1t[:])

                wiAt = wApool.tile([128, KC, 3 * H], BF16, tag="wiA",
                                   name="wiAt")
                wiA = [wiAt[:, kc] for kc in range(KC)]

                xw0 = xwp.tile([128, KC, 8, RP], F32R, tag="xw",
                               name="xw_0")
                nc.sync.dma_start(out=xw0, in_=xtr[:, :, 0:8, :])
                for kc in range(KC):
                    nc.sync.dma_start(out=wiAt[:, kc], in_=wi0t[:, kc])
                # prefetches for the recurrence loop, behind phase A's needs
                for kc in range(KC):
                    nc.sync.dma_start(out=w0t[:, kc], in_=wt0[:, kc])
                for kc in range(KC):
                    nc.sync.dma_start(out=w1t[:, kc], in_=wt1[:, kc])

                for hb in range(8):  # half-blocks of 8 steps
                    if hb == 0:
                        xw = xw0
                    else:
                        xw = xwp.tile([128, KC, 8, RP], F32R, tag="xw",
                                      name=f"xw_{hb}")
                        nc.sync.dma_start(
                            out=xw, in_=xtr[:, :, hb * 8:(hb + 1) * 8, :])
                    gat = gatp.tile([128, 8, 720], F32R, tag="gat",
                                    name=f"gat_{hb}")
                    for g in range(NG):
                        ps = psA.tile([128, 8, RP], F32, tag="psA",
                                      name=f"psA_{hb}_{g}")
                        for kc in range(KC):
                            nc.tensor.matmul(
                                ps, wiA[kc][:, g * 128:(g + 1) * 128],
                                xw[:, kc],
                                start=(kc == 0), stop=(kc == KC - 1))
                        tau, c = g // 8, g % 8
                        off = tau * 240 + c * 30 if tau < 2 else 480 + c * 30
                        nc.vector.tensor_scalar_add(
                            gat[:, :, off:off + 30], ps[:, :, 0:30],
                            bias1_sb[:, g:g + 1])
                    nc.sync.dma_start(
                        out=gi1d[:, hb * 8:(hb + 1) * 8, :], in_=gat)

            # ================= Fused recurrence loop ====================
            with (
                tc.tile_pool(name="wloop2", bufs=1) as wlp2,
                tc.tile_pool(name="constL", bufs=1) as constL,
                tc.tile_pool(name="gi_pool", bufs=2) as gip,
                tc.tile_pool(name="st_pool", bufs=2) as stp,
                tc.tile_pool(name="ew_pool", bufs=2) as ewp,
                tc.tile_pool(name="psL", bufs=2, space="PSUM") as psL,
            ):
                w2t = wlp2.tile([128, KC, 3 * H], BF16, tag="w2", name="w2t")
                w2 = [w2t[:, kc] for kc in range(KC)]

                identf = constL.tile([128, 128], F32)
                make_identity(nc, identf)
                identr = constL.tile([128, 128], F32R)
                nc.vector.tensor_copy(identr, identf)

                b2rz_sb = constL.tile([128, 480], F32R)
                nc.sync.dma_start(out=b2rz_sb, in_=b2rz[:])
                bhn1_sb = constL.tile([128, 256], F32R)
                nc.sync.dma_start(out=bhn1_sb, in_=bhn1[:])
                bhn2_sb = constL.tile([128, 256], F32R)
                nc.sync.dma_start(out=bhn2_sb, in_=bhn2[:])
                bi1n_sb = constL.tile([128, 256], F32R)
                nc.sync.dma_start(out=bi1n_sb, in_=bi1n[:])

                h1f = stp.tile([128, 240], F32, tag="h1f", name="h1f_init")
                nc.sync.dma_start(out=h1f, in_=h1f0[:])
                h2f = stp.tile([128, 240], F32, tag="h2f", name="h2f_init")
                nc.sync.dma_start(out=h2f, in_=h2f0[:])
                h1b = stp.tile([128, 240], BF16, tag="h1b", name="h1b_init")
                nc.sync.dma_start(out=h1b, in_=h1b0[:])
                h2b = stp.tile([128, 240], BF16, tag="h2b", name="h2b_init")
                nc.sync.dma_start(out=h2b, in_=h2b0[:])

                def load_gi(b):
                    t = gip.tile([128, 2, 720], F32R, tag="gw", name=f"gw_{b}")
                    nc.sync.dma_start(out=t, in_=gi1d[:, b * 2:(b + 1) * 2, :])
                    return t

                gtiles = [load_gi(0), load_gi(1)]

                # wh1 weights load at the transition (wi1/wh0 were
                # prefetched during phase A); first consumed ~2 iterations in
                for kc in range(KC):
                    nc.sync.dma_start(out=w2t[:, kc], in_=wt2[:, kc])

                def rec_mms(dst_rz, dst_n, wts, mov, kcs, stop_rz, stop_n):
                    """Gate matmuls for one layer pass: rz slices into
                    dst_rz (480 wide), n slices into dst_n (240 wide)."""
                    last = kcs[-1]
                    for c in range(8):
                        for tau in range(3):
                            g = tau * 8 + c
                            if tau < 2:
                                dst = dst_rz[:, tau * 240 + c * 30:
                                             tau * 240 + c * 30 + 30]
                                stop_k = last if stop_rz else -1
                            else:
                                dst = dst_n[:, c * 30:c * 30 + 30]
                                stop_k = last if stop_n else -1
                            for kc in kcs:
                                nc.tensor.matmul(
                                    dst,
                                    wts[kc][:, g * 128:(g + 1) * 128],
                                    mov[:, kc * 30:(kc + 1) * 30],
                                    start=False,
                                    stop=(kc == stop_k))

                def elementwise(lab, s, hf, Trz, Tn, hfp, ginA, hf_new, hb_new):
                    """GRU combine for h-chunk half hf (cols hf*120..+120)."""
                    lo = hf * 120
                    t = lambda nm: ewp.tile(
                        [128, 120], F32, tag=f"{nm}{hf}{lab}",
                        name=f"{nm}_{lab}_{s}_{hf}")
                    rz = ewp.tile([128, 2, 120], F32, tag=f"rz{hf}{lab}",
                                  name=f"rz_{lab}_{s}_{hf}")
                    nc.scalar.activation(
                        rz,
                        Trz.rearrange("p (t x) -> p t x", t=2)[:, :, lo:lo + 120],
                        AF.Sigmoid)
                    oz = t("oz")
                    nc.vector.tensor_scalar(oz, rz[:, 1], -1.0, 1.0,
                                            OP.mult, OP.add)
                    bz = t("bz")
                    nc.vector.tensor_mul(bz, rz[:, 1], hfp[:, lo:lo + 120])
                    t1 = t("t1")
                    nc.vector.tensor_mul(t1, rz[:, 0], Tn[:, lo:lo + 120])
                    npre = t("np")
                    nc.vector.tensor_add(npre, t1, ginA)
                    nn = t("nn")
                    nc.scalar.activation(nn, npre, AF.Tanh)
                    aa = t("aa")
                    nc.vector.tensor_mul(aa, nn, oz)
                    nc.vector.tensor_add(hf_new[:, lo:lo + 120], aa, bz)
                    nc.vector.tensor_add(hb_new[:, lo:lo + 120], aa, bz)

                # Software pipeline: iteration i runs L1 matmuls of
                # step i and L2 matmuls of step i-1, so each elementwise
                # chain has a full matmul section of PE work to hide under.
                T1s, T2s = {}, {}
                h1 = {-1: (h1f, h1b)}
                h2 = {-1: (h2f, h2b)}

                for it in range(steps + 1):
                    sL1, sL2 = it, it - 1

                    if sL1 < steps:
                        b, j = divmod(sL1, 2)
                        T1 = psL.tile([128, 480], F32, tag="T1",
                                      name=f"T1_{sL1}")
                        T2 = psL.tile([128, 512], F32, tag="T2",
                                      name=f"T2_{sL1}")
                        T1s[sL1], T2s[sL1] = T1, T2
                        g = gtiles[b]
                        nc.tensor.matmul(T1, identr, g[:, j, 0:480],
                                         start=True, stop=False)
                        nc.tensor.matmul(T2[:, 0:256], identr, bhn1_sb,
                                         start=True, stop=False)
                        rec_mms(T1, T2[:, 0:240], w0, h1[sL1 - 1][1],
                                list(range(KC)), True, True)

                        # E1(sL1)
                        h1f_new = stp.tile([128, 240], F32, tag="h1f",
                                           name=f"h1f_{sL1}")
                        h1b_new = stp.tile([128, 240], BF16, tag="h1b",
                                           name=f"h1b_{sL1}")
                        for hf in range(2):
                            elementwise(
                                "a", sL1, hf, T1, T2[:, 0:240],
                                h1[sL1 - 1][0],
                                g[:, j, 480 + hf * 120:480 + hf * 120 + 120],
                                h1f_new, h1b_new)
                        h1[sL1] = (h1f_new, h1b_new)

                        if j == 0 and b + 2 < steps // 2:
                            gtiles.append(load_gi(b + 2))

                    if sL2 >= 0:
                        T3 = psL.tile([128, 480], F32, tag="T3",
                                      name=f"T3_{sL2}")
                        T4 = psL.tile([128, 256], F32, tag="T4",
                                      name=f"T4_{sL2}")
                        T2p = T2s.pop(sL2)
                        nc.tensor.matmul(T3, identr, b2rz_sb,
                                         start=True, stop=False)
                        nc.tensor.matmul(T2p[:, 256:512], identr, bhn2_sb,
                                         start=True, stop=False)
                        nc.tensor.matmul(T4, identr, bi1n_sb,
                                         start=True, stop=False)
                        # wi1 first (h1b(sL2) is a full iteration old), then
                        # wh1 (its h2b dep is the freshest elementwise)
                        rec_mms(T3, T4[:, 0:240], w1, h1[sL2][1],
                                list(range(KC)), False, True)
                        rec_mms(T3, T2p[:, 256:496], w2, h2[sL2 - 1][1],
                                list(range(KC)), True, True)

                        # E2(sL2)
                        h2f_new = stp.tile([128, 240], F32, tag="h2f",
                                           name=f"h2f_{sL2}")
                        h2b_new = stp.tile([128, 240], BF16, tag="h2b",
                                           name=f"h2b_{sL2}")
                        for hf in range(2):
                            elementwise(
                                "b", sL2, hf, T3, T2p[:, 256:496],
                                h2[sL2 - 1][0],
                                T4[:, hf * 120:hf * 120 + 120],
                                h2f_new, h2b_new)
                        h2[sL2] = (h2f_new, h2b_new)
                        nc.sync.dma_start(out=out[:, sL2, :], in_=h2f_new)

                        h1.pop(sL2 - 1, None)
                        h2.pop(sL2 - 2, None)
                        T1s.pop(sL2, None)

    nc.finalize()
    return nc


def ode_traj(w1, b1, w2, b2, w3, b3):
    """RK4 trajectory of the ODE, mirroring the reference exactly (fp32)."""
    w1t = w1.T.astype(np.float32)
    w2t = w2.T.astype(np.float32)
    w3t = w3.T.astype(np.float32)

    def f(h):
        a = np.tanh(h @ w1t + b1)
        a = np.tanh(a @ w2t + b2)
        return a @ w3t + b3

    dt = np.float32((1.0 / NSEG) / SUB)
    h = np.zeros((2, H), np.float32)
    traj = []
    for _ in range(NSEG):
        for _ in range(SUB):
            k1 = f(h)
            k2 = f(h + np.float32(0.5) * dt * k1)
            k3 = f(h + np.float32(0.5) * dt * k2)
            k4 = f(h + dt * k3)
            h = h + (dt / np.float32(6.0)) * (k1 + np.float32(2.0) * k2
                                              + np.float32(2.0) * k3 + k4)
        traj.append(h.copy())
    return np.stack(traj)  # (NSEG, 2, H)


def _bc_runs(per_gate, width):
    """[G] gate-vector -> [128, width] broadcast over 30 runs; G = n*128,
    cols laid out (chunk, run) with zero padding to `width`."""
    nchunk = per_gate.size // 128
    a = per_gate.reshape(nchunk, 128)  # [chunk, p]
    o = np.zeros((128, width), np.float32)
    o[:, :nchunk * 30] = np.repeat(
        a.T[:, :, None], 30, axis=2).reshape(128, nchunk * 30)
    return o


def make_in_maps(x, w1, b1, w2, b2, w3, b3, wi0, wh0, bi0, bh0,
                 wi1, wh1, bi1, bh1, cores=NCORES):
    traj = ode_traj(w1, b1, w2, b2, w3, b3)
    bf = ml_dtypes.bfloat16

    xtr = np.zeros((128, KC, S, RP), np.float32)
    # xtr[p, kc, s, r] = x[s, r, kc*128+p]
    xtr[:, :, :, :T] = np.ascontiguousarray(
        x.reshape(S, T, KC, 128).transpose(3, 2, 0, 1))

    bias1 = np.concatenate([bi0[:2 * H] + bh0[:2 * H], bi0[2 * H:]])

    shared = {
        "xtr": xtr,
        "wi0t": np.ascontiguousarray(
            wi0.T.reshape(KC, 128, 3 * H).transpose(1, 0, 2)).astype(bf),
        "wt0": np.ascontiguousarray(
            wh0.T.reshape(KC, 128, 3 * H).transpose(1, 0, 2)).astype(bf),
        "wt1": np.ascontiguousarray(
            wi1.T.reshape(KC, 128, 3 * H).transpose(1, 0, 2)).astype(bf),
        "wt2": np.ascontiguousarray(
            wh1.T.reshape(KC, 128, 3 * H).transpose(1, 0, 2)).astype(bf),
        "bias1t": np.ascontiguousarray(bias1.reshape(NG, 128).T),
        "b2rz": _bc_runs((bi1 + bh1)[:2 * H], 480),
        "bhn1": _bc_runs(bh0[2 * H:], 256),
        "bhn2": _bc_runs(bh1[2 * H:], 256),
        "bi1n": _bc_runs(bi1[2 * H:], 256),
    }
    in_maps = []
    for i in range(cores):
        m = dict(shared)
        for li, nm in ((0, "h1"), (1, "h2")):
            hf = np.repeat(traj[i, li].reshape(KC, 128).T[:, :, None],
                           30, axis=2).reshape(128, 240)
            m[f"{nm}f0"] = np.ascontiguousarray(hf)
            m[f"{nm}b0"] = np.ascontiguousarray(hf).astype(bf)
        in_maps.append(m)
    return in_maps


_NC_CACHE = {}


def _get_nc(steps=S):
    if steps not in _NC_CACHE:
        _NC_CACHE[steps] = build_nc(steps)
    return _NC_CACHE[steps]


def run_cores(inputs, steps=S, cores=NCORES, **run_kwargs):
    in_maps = make_in_maps(cores=cores, **inputs)
    nc = _get_nc(steps)
    return run_bass_kernel_spmd(nc, in_maps, core_ids=list(range(cores)),
                                **run_kwargs)


def kernel(x, w1, b1, w2, b2, w3, b3, wi0, wh0, bi0, bh0,
           wi1, wh1, bi1, bh1):
    args = dict(x=x, w1=w1, b1=b1, w2=w2, b2=b2, w3=w3, b3=b3,
                wi0=wi0, wh0=wh0, bi0=bi0, bh0=bh0,
                wi1=wi1, wh1=wh1, bi1=bi1, bh1=bh1)
    args = {k: np.asarray(v, np.float32) for k, v in args.items()}
    res = run_cores(args, steps=S, cores=NCORES)
    B = 64
    full = np.empty((B, T * NCORES, H), np.float32)
    for i in range(NCORES):
        o = np.asarray(res.results[i]["out"], np.float32)
        # out[p, s, c*30+t] -> full[s, t*8+i, c*128+p]
        full[:, i::NCORES, :] = o.reshape(
            128, S, KC, 30).transpose(1, 3, 2, 0).reshape(B, T, H)
    return full


# revision 11
# speedup vs baseline: 3.2599x; 1.0463x over previous
"""ODE-RNN Trainium2 kernel, v2 (gates-major fused recurrence).

out[b, t*8+i, :] = 2-layer GRU (H=1024) over the batch dim (64 steps) of
sequence t (30 sequences), init hiddens from an RK4 ODE trajectory
(8 grid points).  Core i handles the 30 runs with init traj[i].

Per-core structure:
  Phase A: gi1 = x @ wi0.T + bias, gates-major in f32r, written to DRAM
           as [128, 64 steps, 720]; free cols = (tau{r,z}, chunk, run)
           for [0:480], n-gate (chunk, run) for [480:720].
  Loop (64 steps, both layers fused per step): recurrent matmuls are
           gates-major: out tile [128 gates, 30 runs] in PSUM,
           stationary = bf16 weight tile [128 k, 128 gates], moving =
           bf16 state [128 k, 30 runs] (bf16 moving -> 1 cycle/row).
           gi / biases enter PSUM via f32r identity-matmuls (>=256
           wide).  Layer-2's wi1 matmuls accumulate into the same PSUM
           group as wh1 (rz) or a dedicated bank (n), so there is no
           dense gi2 phase and no h1-state saving.  Elementwise is f32
           on ACT/DVE, split into h-chunk halves to pipeline under the
           PE stream.  State kept twice: f32 master + bf16 PE copy.
"""

import numpy as np

try:
    import concourse.bass as bass  # noqa: F401
except ImportError:  # pragma: no cover
    import sys
    sys.path.insert(0, "/opt/trn_rl_repo")
    import concourse.bass as bass  # noqa: F401

import ml_dtypes
import concourse.mybir as mybir
import concourse.tile as tile
from concourse import bacc
from concourse.bass_utils import run_bass_kernel_spmd
from concourse.masks import make_identity

F32 = mybir.dt.float32
F32R = mybir.dt.float32r
BF16 = mybir.dt.bfloat16
AF = mybir.ActivationFunctionType
OP = mybir.AluOpType

H = 1024
KC = 8          # k chunks of 128
NG = 24         # gate tiles (tau*8 + c)
R = 30          # runs per core (exact, no padding)
RP = 32         # padded runs used in phase A only (psum rows >= 256)
S = 64          # steps (batch-as-sequence)
NSEG = 8
SUB = 4
NCORES = 8
T = 30          # sequences


def build_nc(steps=S):
    nc = bacc.Bacc()

    xtr = nc.declare_dram_parameter("xtr", [128, KC, S, RP], F32R, isOutput=False)
    wi0t = nc.declare_dram_parameter("wi0t", [128, KC, 3 * H], BF16, isOutput=False)
    wt0 = nc.declare_dram_parameter("wt0", [128, KC, 3 * H], BF16, isOutput=False)
    wt1 = nc.declare_dram_parameter("wt1", [128, KC, 3 * H], BF16, isOutput=False)
    wt2 = nc.declare_dram_parameter("wt2", [128, KC, 3 * H], BF16, isOutput=False)
    bias1t = nc.declare_dram_parameter("bias1t", [128, NG], F32, isOutput=False)
    b2rz = nc.declare_dram_parameter("b2rz", [128, 480], F32R, isOutput=False)
    bhn1 = nc.declare_dram_parameter("bhn1", [128, 256], F32R, isOutput=False)
    bhn2 = nc.declare_dram_parameter("bhn2", [128, 256], F32R, isOutput=False)
    bi1n = nc.declare_dram_parameter("bi1n", [128, 256], F32R, isOutput=False)
    h1f0 = nc.declare_dram_parameter("h1f0", [128, 240], F32, isOutput=False)
    h2f0 = nc.declare_dram_parameter("h2f0", [128, 240], F32, isOutput=False)
    h1b0 = nc.declare_dram_parameter("h1b0", [128, 240], BF16, isOutput=False)
    h2b0 = nc.declare_dram_parameter("h2b0", [128, 240], BF16, isOutput=False)
    out = nc.declare_dram_parameter("out", [128, S, 240], F32, isOutput=True)

    gi1d = nc.dram_tensor("gi1d", [128, S, 720], F32R)

    with tile.TileContext(nc) as tc:
        with tc.tile_pool(name="wloop", bufs=1) as wlp:
            # layer-1/2 recurrent weights, prefetched during phase A
            # (emitted after phase A's own input DMAs; chunked so early
            # consumers unblock as chunks land)
            w0t = wlp.tile([128, KC, 3 * H], BF16, tag="w0", name="w0t")
            w0 = [w0t[:, kc] for kc in range(KC)]
            w1t = wlp.tile([128, KC, 3 * H], BF16, tag="w1", name="w1t")
            w1 = [w1t[:, kc] for kc in range(KC)]

            # ============= Phase A: gi1 (f32r, gates-major) =============
            with (
                tc.tile_pool(name="wApool", bufs=1) as wApool,
                tc.tile_pool(name="xw_pool", bufs=2) as xwp,
                tc.tile_pool(name="gat_pool", bufs=2) as gatp,
                tc.tile_pool(name="psA", bufs=4, space="PSUM") as psA,
                tc.tile_pool(name="constA", bufs=1) as constA,
            ):
                bias1_sb = constA.tile([128, NG], F32)
                nc.sync.dma_start(out=bias1_sb, in_=bias1t[:])

                wiAt = wApool.tile([128, KC, 3 * H], BF16, tag="wiA",
                                   name="wiAt")
                wiA = [wiAt[:, kc] for kc in range(KC)]

                xw0 = xwp.tile([128, KC, 8, RP], F32R, tag="xw",
                               name="xw_0")
                nc.sync.dma_start(out=xw0, in_=xtr[:, :, 0:8, :])
                for kc in range(KC):
                    nc.sync.dma_start(out=wiAt[:, kc], in_=wi0t[:, kc])
                # loop-weight prefetch chunks, doled out between the
                # half-block sections so they never starve phase A's DMAs
                prefetch = [(w0t, wt0, kc) for kc in range(KC)] + \
                           [(w1t, wt1, kc) for kc in range(KC)]

                for hb in range(8):  # half-blocks of 8 steps
                    if hb == 0:
                        xw = xw0
                    else:
                        xw = xwp.tile([128, KC, 8, RP], F32R, tag="xw",
                                      name=f"xw_{hb}")
                        nc.sync.dma_start(
                            out=xw, in_=xtr[:, :, hb * 8:(hb + 1) * 8, :])
                    for _ in range(2):
                        if prefetch:
                            dst, srcp, kc = prefetch.pop(0)
                            nc.sync.dma_start(out=dst[:, kc], in_=srcp[:, kc])
                    gat = gatp.tile([128, 8, 720], F32R, tag="gat",
                                    name=f"gat_{hb}")
                    for g in range(NG):
                        ps = psA.tile([128, 8, RP], F32, tag="psA",
                                      name=f"psA_{hb}_{g}")
                        for kc in range(KC):
                            nc.tensor.matmul(
                                ps, wiA[kc][:, g * 128:(g + 1) * 128],
                                xw[:, kc],
                                start=(kc == 0), stop=(kc == KC - 1))
                        tau, c = g // 8, g % 8
                        off = tau * 240 + c * 30 if tau < 2 else 480 + c * 30
                        nc.vector.tensor_scalar_add(
                            gat[:, :, off:off + 30], ps[:, :, 0:30],
                            bias1_sb[:, g:g + 1])
                    nc.sync.dma_start(
                        out=gi1d[:, hb * 8:(hb + 1) * 8, :], in_=gat)

            # ================= Fused recurrence loop ====================
            with (
                tc.tile_pool(name="wloop2", bufs=1) as wlp2,
                tc.tile_pool(name="constL", bufs=1) as constL,
                tc.tile_pool(name="gi_pool", bufs=2) as gip,
                tc.tile_pool(name="st_pool", bufs=2) as stp,
                tc.tile_pool(name="ew_pool", bufs=2) as ewp,
                tc.tile_pool(name="psL", bufs=2, space="PSUM") as psL,
            ):
                w2t = wlp2.tile([128, KC, 3 * H], BF16, tag="w2", name="w2t")
                w2 = [w2t[:, kc] for kc in range(KC)]

                identf = constL.tile([128, 128], F32)
                make_identity(nc, identf)
                identr = constL.tile([128, 128], F32R)
                nc.vector.tensor_copy(identr, identf)

                b2rz_sb = constL.tile([128, 480], F32R)
                nc.sync.dma_start(out=b2rz_sb, in_=b2rz[:])
                bhn1_sb = constL.tile([128, 256], F32R)
                nc.sync.dma_start(out=bhn1_sb, in_=bhn1[:])
                bhn2_sb = constL.tile([128, 256], F32R)
                nc.sync.dma_start(out=bhn2_sb, in_=bhn2[:])
                bi1n_sb = constL.tile([128, 256], F32R)
                nc.sync.dma_start(out=bi1n_sb, in_=bi1n[:])

                h1f = stp.tile([128, 240], F32, tag="h1f", name="h1f_init")
                nc.sync.dma_start(out=h1f, in_=h1f0[:])
                h2f = stp.tile([128, 240], F32, tag="h2f", name="h2f_init")
                nc.sync.dma_start(out=h2f, in_=h2f0[:])
                h1b = stp.tile([128, 240], BF16, tag="h1b", name="h1b_init")
                nc.sync.dma_start(out=h1b, in_=h1b0[:])
                h2b = stp.tile([128, 240], BF16, tag="h2b", name="h2b_init")
                nc.sync.dma_start(out=h2b, in_=h2b0[:])

                def load_gi(b):
                    t = gip.tile([128, 2, 720], F32R, tag="gw", name=f"gw_{b}")
                    nc.sync.dma_start(out=t, in_=gi1d[:, b * 2:(b + 1) * 2, :])
                    return t

                gtiles = [load_gi(0)]
                # wh1 weights load at the transition (wi1/wh0 were
                # prefetched during phase A); first consumed ~2 iterations in
                for kc in range(KC):
                    nc.sync.dma_start(out=w2t[:, kc], in_=wt2[:, kc])
                gtiles.append(load_gi(1))

                def rec_mms(dst_rz, dst_n, wts, mov, kcs, stop_rz, stop_n):
                    """Gate matmuls for one layer pass: rz slices into
                    dst_rz (480 wide), n slices into dst_n (240 wide)."""
                    last = kcs[-1]
                    for c in range(8):
                        for tau in range(3):
                            g = tau * 8 + c
                            if tau < 2:
                                dst = dst_rz[:, tau * 240 + c * 30:
                                             tau * 240 + c * 30 + 30]
                                stop_k = last if stop_rz else -1
                            else:
                                dst = dst_n[:, c * 30:c * 30 + 30]
                                stop_k = last if stop_n else -1
                            for kc in kcs:
                                nc.tensor.matmul(
                                    dst,
                                    wts[kc][:, g * 128:(g + 1) * 128],
                                    mov[:, kc * 30:(kc + 1) * 30],
                                    start=False,
                                    stop=(kc == stop_k))

                def elementwise(lab, s, hf, Trz, Tn, hfp, ginA, hf_new, hb_new):
                    """GRU combine for h-chunk half hf (cols hf*120..+120)."""
                    lo = hf * 120
                    t = lambda nm: ewp.tile(
                        [128, 120], F32, tag=f"{nm}{hf}{lab}",
                        name=f"{nm}_{lab}_{s}_{hf}")
                    rz = ewp.tile([128, 2, 120], F32, tag=f"rz{hf}{lab}",
                                  name=f"rz_{lab}_{s}_{hf}")
                    nc.scalar.activation(
                        rz,
                        Trz.rearrange("p (t x) -> p t x", t=2)[:, :, lo:lo + 120],
                        AF.Sigmoid)
                    oz = t("oz")
                    nc.vector.tensor_scalar(oz, rz[:, 1], -1.0, 1.0,
                                            OP.mult, OP.add)
                    bz = t("bz")
                    nc.vector.tensor_mul(bz, rz[:, 1], hfp[:, lo:lo + 120])
                    t1 = t("t1")
                    nc.vector.tensor_mul(t1, rz[:, 0], Tn[:, lo:lo + 120])
                    npre = t("np")
                    nc.vector.tensor_add(npre, t1, ginA)
                    nn = t("nn")
                    nc.scalar.activation(nn, npre, AF.Tanh)
                    aa = t("aa")
                    nc.vector.tensor_mul(aa, nn, oz)
                    nc.vector.tensor_add(hf_new[:, lo:lo + 120], aa, bz)
                    nc.vector.tensor_add(hb_new[:, lo:lo + 120], aa, bz)

                # Software pipeline: iteration i runs L1 matmuls of
                # step i and L2 matmuls of step i-1, so each elementwise
                # chain has a full matmul section of PE work to hide under.
                T1s, T2s = {}, {}
                h1 = {-1: (h1f, h1b)}
                h2 = {-1: (h2f, h2b)}

                for it in range(steps + 1):
                    sL1, sL2 = it, it - 1

                    if sL1 < steps:
                        b, j = divmod(sL1, 2)
                        T1 = psL.tile([128, 480], F32, tag="T1",
                                      name=f"T1_{sL1}")
                        T2 = psL.tile([128, 512], F32, tag="T2",
                                      name=f"T2_{sL1}")
                        T1s[sL1], T2s[sL1] = T1, T2
                        g = gtiles[b]
                        nc.tensor.matmul(T1, identr, g[:, j, 0:480],
                                         start=True, stop=False)
                        nc.tensor.matmul(T2[:, 0:256], identr, bhn1_sb,
                                         start=True, stop=False)
                        rec_mms(T1, T2[:, 0:240], w0, h1[sL1 - 1][1],
                                list(range(KC)), True, True)

                        # E1(sL1)
                        h1f_new = stp.tile([128, 240], F32, tag="h1f",
                                           name=f"h1f_{sL1}")
                        h1b_new = stp.tile([128, 240], BF16, tag="h1b",
                                           name=f"h1b_{sL1}")
                        for hf in range(2):
                            elementwise(
                                "a", sL1, hf, T1, T2[:, 0:240],
                                h1[sL1 - 1][0],
                                g[:, j, 480 + hf * 120:480 + hf * 120 + 120],
                                h1f_new, h1b_new)
                        h1[sL1] = (h1f_new, h1b_new)

                        if j == 0 and b + 2 < steps // 2:
                            gtiles.append(load_gi(b + 2))

                    if sL2 >= 0:
                        T3 = psL.tile([128, 480], F32, tag="T3",
                                      name=f"T3_{sL2}")
                        T4 = psL.tile([128, 256], F32, tag="T4",
                                      name=f"T4_{sL2}")
                        T2p = T2s.pop(sL2)
                        nc.tensor.matmul(T3, identr, b2rz_sb,
                                         start=True, stop=False)
                        nc.tensor.matmul(T2p[:, 256:512], identr, bhn2_sb,
                                         start=True, stop=False)
                        nc.tensor.matmul(T4, identr, bi1n_sb,
                                         start=True, stop=False)
                        # wi1 first (h1b(sL2) is a full iteration old), then
                        # wh1 (its h2b dep is the freshest elementwise)
                        rec_mms(T3, T4[:, 0:240], w1, h1[sL2][1],
                                list(range(KC)), False, True)
                        rec_mms(T3, T2p[:, 256:496], w2, h2[sL2 - 1][1],
                                list(range(KC)), True, True)

                        # E2(sL2)
                        h2f_new = stp.tile([128, 240], F32, tag="h2f",
                                           name=f"h2f_{sL2}")
                        h2b_new = stp.tile([128, 240], BF16, tag="h2b",
                                           name=f"h2b_{sL2}")
                        for hf in range(2):
                            elementwise(
                                "b", sL2, hf, T3, T2p[:, 256:496],
                                h2[sL2 - 1][0],
                                T4[:, hf * 120:hf * 120 + 120],
                                h2f_new, h2b_new)
                        h2[sL2] = (h2f_new, h2b_new)
                        nc.sync.dma_start(out=out[:, sL2, :], in_=h2f_new)

                        h1.pop(sL2 - 1, None)
                        h2.pop(sL2 - 2, None)
                        T1s.pop(sL2, None)

    nc.finalize()
    return nc


def ode_traj(w1, b1, w2, b2, w3, b3):
    """RK4 trajectory of the ODE, mirroring the reference exactly (fp32)."""
    w1t = w1.T.astype(np.float32)
    w2t = w2.T.astype(np.float32)
    w3t = w3.T.astype(np.float32)

    def f(h):
        a = np.tanh(h @ w1t + b1)
        a = np.tanh(a @ w2t + b2)
        return a @ w3t + b3

    dt = np.float32((1.0 / NSEG) / SUB)
    h = np.zeros((2, H), np.float32)
    traj = []
    for _ in range(NSEG):
        for _ in range(SUB):
            k1 = f(h)
            k2 = f(h + np.float32(0.5) * dt * k1)
            k3 = f(h + np.float32(0.5) * dt * k2)
            k4 = f(h + dt * k3)
            h = h + (dt / np.float32(6.0)) * (k1 + np.float32(2.0) * k2
                                              + np.float32(2.0) * k3 + k4)
        traj.append(h.copy())
    return np.stack(traj)  # (NSEG, 2, H)


def _bc_runs(per_gate, width):
    """[G] gate-vector -> [128, width] broadcast over 30 runs; G = n*128,
    cols laid out (chunk, run) with zero padding to `width`."""
    nchunk = per_gate.size // 128
    a = per_gate.reshape(nchunk, 128)  # [chunk, p]
    o = np.zeros((128, width), np.float32)
    o[:, :nchunk * 30] = np.repeat(
        a.T[:, :, None], 30, axis=2).reshape(128, nchunk * 30)
    return o


def make_in_maps(x, w1, b1, w2, b2, w3, b3, wi0, wh0, bi0, bh0,
                 wi1, wh1, bi1, bh1, cores=NCORES):
    traj = ode_traj(w1, b1, w2, b2, w3, b3)
    bf = ml_dtypes.bfloat16

    xtr = np.zeros((128, KC, S, RP), np.float32)
    # xtr[p, kc, s, r] = x[s, r, kc*128+p]
    xtr[:, :, :, :T] = np.ascontiguousarray(
        x.reshape(S, T, KC, 128).transpose(3, 2, 0, 1))

    bias1 = np.concatenate([bi0[:2 * H] + bh0[:2 * H], bi0[2 * H:]])

    shared = {
        "xtr": xtr,
        "wi0t": np.ascontiguousarray(
            wi0.T.reshape(KC, 128, 3 * H).transpose(1, 0, 2)).astype(bf),
        "wt0": np.ascontiguousarray(
            wh0.T.reshape(KC, 128, 3 * H).transpose(1, 0, 2)).astype(bf),
        "wt1": np.ascontiguousarray(
            wi1.T.reshape(KC, 128, 3 * H).transpose(1, 0, 2)).astype(bf),
        "wt2": np.ascontiguousarray(
            wh1.T.reshape(KC, 128, 3 * H).transpose(1, 0, 2)).astype(bf),
        "bias1t": np.ascontiguousarray(bias1.reshape(NG, 128).T),
        "b2rz": _bc_runs((bi1 + bh1)[:2 * H], 480),
        "bhn1": _bc_runs(bh0[2 * H:], 256),
        "bhn2": _bc_runs(bh1[2 * H:], 256),
        "bi1n": _bc_runs(bi1[2 * H:], 256),
    }
    in_maps = []
    for i in range(cores):
        m = dict(shared)
        for li, nm in ((0, "h1"), (1, "h2")):
            hf = np.repeat(traj[i, li].reshape(KC, 128).T[:, :, None],
                           30, axis=2).reshape(128, 240)
            m[f"{nm}f0"] = np.ascontiguousarray(hf)
            m[f"{nm}b0"] = np.ascontiguousarray(hf).astype(bf)
        in_maps.append(m)
    return in_maps


_NC_CACHE = {}


def _get_nc(steps=S):
    if steps not in _NC_CACHE:
        _NC_CACHE[steps] = build_nc(steps)
    return _NC_CACHE[steps]


def run_cores(inputs, steps=S, cores=NCORES, **run_kwargs):
    in_maps = make_in_maps(cores=cores, **inputs)
    nc = _get_nc(steps)
    return run_bass_kernel_spmd(nc, in_maps, core_ids=list(range(cores)),
                                **run_kwargs)


def kernel(x, w1, b1, w2, b2, w3, b3, wi0, wh0, bi0, bh0,
           wi1, wh1, bi1, bh1):
    args = dict(x=x, w1=w1, b1=b1, w2=w2, b2=b2, w3=w3, b3=b3,
                wi0=wi0, wh0=wh0, bi0=bi0, bh0=bh0,
                wi1=wi1, wh1=wh1, bi1=bi1, bh1=bh1)
    args = {k: np.asarray(v, np.float32) for k, v in args.items()}
    res = run_cores(args, steps=S, cores=NCORES)
    B = 64
    full = np.empty((B, T * NCORES, H), np.float32)
    for i in range(NCORES):
        o = np.asarray(res.results[i]["out"], np.float32)
        # out[p, s, c*30+t] -> full[s, t*8+i, c*128+p]
        full[:, i::NCORES, :] = o.reshape(
            128, S, KC, 30).transpose(1, 3, 2, 0).reshape(B, T, H)
    return full


# revision 12
# speedup vs baseline: 3.3072x; 1.0145x over previous
"""ODE-RNN Trainium2 kernel, v2 (gates-major fused recurrence).

out[b, t*8+i, :] = 2-layer GRU (H=1024) over the batch dim (64 steps) of
sequence t (30 sequences), init hiddens from an RK4 ODE trajectory
(8 grid points).  Core i handles the 30 runs with init traj[i].

Per-core structure:
  Phase A: gi1 = x @ wi0.T + bias, gates-major in f32r, written to DRAM
           as [128, 64 steps, 720]; free cols = (tau{r,z}, chunk, run)
           for [0:480], n-gate (chunk, run) for [480:720].
  Loop (64 steps, both layers fused per step): recurrent matmuls are
           gates-major: out tile [128 gates, 30 runs] in PSUM,
           stationary = bf16 weight tile [128 k, 128 gates], moving =
           bf16 state [128 k, 30 runs] (bf16 moving -> 1 cycle/row).
           gi / biases enter PSUM via f32r identity-matmuls (>=256
           wide).  Layer-2's wi1 matmuls accumulate into the same PSUM
           group as wh1 (rz) or a dedicated bank (n), so there is no
           dense gi2 phase and no h1-state saving.  Elementwise is f32
           on ACT/DVE, split into h-chunk halves to pipeline under the
           PE stream.  State kept twice: f32 master + bf16 PE copy.
"""

import numpy as np

try:
    import concourse.bass as bass  # noqa: F401
except ImportError:  # pragma: no cover
    import sys
    sys.path.insert(0, "/opt/trn_rl_repo")
    import concourse.bass as bass  # noqa: F401

import ml_dtypes
import concourse.mybir as mybir
import concourse.tile as tile
from concourse import bacc
from concourse.bass_utils import run_bass_kernel_spmd
from concourse.masks import make_identity

F32 = mybir.dt.float32
F32R = mybir.dt.float32r
BF16 = mybir.dt.bfloat16
AF = mybir.ActivationFunctionType
OP = mybir.AluOpType

H = 1024
KC = 8          # k chunks of 128
NG = 24         # gate tiles (tau*8 + c)
R = 30          # runs per core (exact, no padding)
RP = 32         # padded runs used in phase A only (psum rows >= 256)
S = 64          # steps (batch-as-sequence)
NSEG = 8
SUB = 4
NCORES = 8
T = 30          # sequences


def build_nc(steps=S):
    nc = bacc.Bacc()

    xtr = nc.declare_dram_parameter("xtr", [128, KC, S, 30], BF16, isOutput=False)
    wi0t = nc.declare_dram_parameter("wi0t", [128, KC, 3 * H], BF16, isOutput=False)
    wt0 = nc.declare_dram_parameter("wt0", [128, KC, 3 * H], BF16, isOutput=False)
    wt1 = nc.declare_dram_parameter("wt1", [128, KC, 3 * H], BF16, isOutput=False)
    wt2 = nc.declare_dram_parameter("wt2", [128, KC, 3 * H], BF16, isOutput=False)
    bias1t = nc.declare_dram_parameter("bias1t", [128, NG], F32, isOutput=False)
    b2rz = nc.declare_dram_parameter("b2rz", [128, 480], F32R, isOutput=False)
    bhn1 = nc.declare_dram_parameter("bhn1", [128, 256], F32R, isOutput=False)
    bhn2 = nc.declare_dram_parameter("bhn2", [128, 256], F32R, isOutput=False)
    bi1n = nc.declare_dram_parameter("bi1n", [128, 256], F32R, isOutput=False)
    h1f0 = nc.declare_dram_parameter("h1f0", [128, 240], F32, isOutput=False)
    h2f0 = nc.declare_dram_parameter("h2f0", [128, 240], F32, isOutput=False)
    h1b0 = nc.declare_dram_parameter("h1b0", [128, 240], BF16, isOutput=False)
    h2b0 = nc.declare_dram_parameter("h2b0", [128, 240], BF16, isOutput=False)
    out = nc.declare_dram_parameter("out", [128, S, 240], F32, isOutput=True)

    gi1d = nc.dram_tensor("gi1d", [128, S, 720], F32R)

    with tile.TileContext(nc) as tc:
        with tc.tile_pool(name="wloop", bufs=1) as wlp:
            # layer-1/2 recurrent weights, prefetched during phase A
            # (emitted after phase A's own input DMAs; chunked so early
            # consumers unblock as chunks land)
            w0t = wlp.tile([128, KC, 3 * H], BF16, tag="w0", name="w0t")
            w0 = [w0t[:, kc] for kc in range(KC)]
            w1t = wlp.tile([128, KC, 3 * H], BF16, tag="w1", name="w1t")
            w1 = [w1t[:, kc] for kc in range(KC)]

            # ============= Phase A: gi1 (f32r, gates-major) =============
            with (
                tc.tile_pool(name="wApool", bufs=1) as wApool,
                tc.tile_pool(name="xw_pool", bufs=2) as xwp,
                tc.tile_pool(name="gat_pool", bufs=2) as gatp,
                tc.tile_pool(name="psA", bufs=4, space="PSUM") as psA,
                tc.tile_pool(name="constA", bufs=1) as constA,
            ):
                bias1_sb = constA.tile([128, NG], F32)
                nc.sync.dma_start(out=bias1_sb, in_=bias1t[:])

                wiAt = wApool.tile([128, KC, 3 * H], BF16, tag="wiA",
                                   name="wiAt")
                wiA = [wiAt[:, kc] for kc in range(KC)]

                xw0 = xwp.tile([128, KC, 8, 30], BF16, tag="xw",
                               name="xw_0")
                nc.sync.dma_start(out=xw0, in_=xtr[:, :, 0:8, :])
                for kc in range(KC):
                    nc.sync.dma_start(out=wiAt[:, kc], in_=wi0t[:, kc])
                # loop-weight prefetch chunks, doled out between the
                # half-block sections so they never starve phase A's DMAs
                prefetch = [(w0t, wt0, kc) for kc in range(KC)] + \
                           [(w1t, wt1, kc) for kc in range(KC)]

                for hb in range(8):  # half-blocks of 8 steps
                    if hb == 0:
                        xw = xw0
                    else:
                        xw = xwp.tile([128, KC, 8, 30], BF16, tag="xw",
                                      name=f"xw_{hb}")
                        nc.sync.dma_start(
                            out=xw, in_=xtr[:, :, hb * 8:(hb + 1) * 8, :])
                    for _ in range(2):
                        if prefetch:
                            dst, srcp, kc = prefetch.pop(0)
                            nc.sync.dma_start(out=dst[:, kc], in_=srcp[:, kc])
                    gat = gatp.tile([128, 8, 720], F32R, tag="gat",
                                    name=f"gat_{hb}")
                    for g in range(NG):
                        ps = psA.tile([128, 8, 30], F32, tag="psA",
                                      name=f"psA_{hb}_{g}")
                        for kc in range(KC):
                            nc.tensor.matmul(
                                ps, wiA[kc][:, g * 128:(g + 1) * 128],
                                xw[:, kc],
                                start=(kc == 0), stop=(kc == KC - 1))
                        tau, c = g // 8, g % 8
                        off = tau * 240 + c * 30 if tau < 2 else 480 + c * 30
                        nc.vector.tensor_scalar_add(
                            gat[:, :, off:off + 30], ps,
                            bias1_sb[:, g:g + 1])
                    nc.sync.dma_start(
                        out=gi1d[:, hb * 8:(hb + 1) * 8, :], in_=gat)

            # ================= Fused recurrence loop ====================
            with (
                tc.tile_pool(name="wloop2", bufs=1) as wlp2,
                tc.tile_pool(name="constL", bufs=1) as constL,
                tc.tile_pool(name="gi_pool", bufs=2) as gip,
                tc.tile_pool(name="st_pool", bufs=2) as stp,
                tc.tile_pool(name="ew_pool", bufs=2) as ewp,
                tc.tile_pool(name="psL", bufs=2, space="PSUM") as psL,
            ):
                w2t = wlp2.tile([128, KC, 3 * H], BF16, tag="w2", name="w2t")
                w2 = [w2t[:, kc] for kc in range(KC)]

                identf = constL.tile([128, 128], F32)
                make_identity(nc, identf)
                identr = constL.tile([128, 128], F32R)
                nc.vector.tensor_copy(identr, identf)

                b2rz_sb = constL.tile([128, 480], F32R)
                nc.sync.dma_start(out=b2rz_sb, in_=b2rz[:])
                bhn1_sb = constL.tile([128, 256], F32R)
                nc.sync.dma_start(out=bhn1_sb, in_=bhn1[:])
                bhn2_sb = constL.tile([128, 256], F32R)
                nc.sync.dma_start(out=bhn2_sb, in_=bhn2[:])
                bi1n_sb = constL.tile([128, 256], F32R)
                nc.sync.dma_start(out=bi1n_sb, in_=bi1n[:])

                h1f = stp.tile([128, 240], F32, tag="h1f", name="h1f_init")
                nc.sync.dma_start(out=h1f, in_=h1f0[:])
                h2f = stp.tile([128, 240], F32, tag="h2f", name="h2f_init")
                nc.sync.dma_start(out=h2f, in_=h2f0[:])
                h1b = stp.tile([128, 240], BF16, tag="h1b", name="h1b_init")
                nc.sync.dma_start(out=h1b, in_=h1b0[:])
                h2b = stp.tile([128, 240], BF16, tag="h2b", name="h2b_init")
                nc.sync.dma_start(out=h2b, in_=h2b0[:])

                def load_gi(b):
                    t = gip.tile([128, 2, 720], F32R, tag="gw", name=f"gw_{b}")
                    nc.sync.dma_start(out=t, in_=gi1d[:, b * 2:(b + 1) * 2, :])
                    return t

                gtiles = [load_gi(0)]
                # wh1 weights load at the transition (wi1/wh0 were
                # prefetched during phase A); first consumed ~2 iterations in
                for kc in range(KC):
                    nc.sync.dma_start(out=w2t[:, kc], in_=wt2[:, kc])
                gtiles.append(load_gi(1))

                def rec_mms(dst_rz, dst_n, wts, mov, kcs, stop_rz, stop_n):
                    """Gate matmuls for one layer pass: rz slices into
                    dst_rz (480 wide), n slices into dst_n (240 wide)."""
                    last = kcs[-1]
                    for c in range(8):
                        for tau in range(3):
                            g = tau * 8 + c
                            if tau < 2:
                                dst = dst_rz[:, tau * 240 + c * 30:
                                             tau * 240 + c * 30 + 30]
                                stop_k = last if stop_rz else -1
                            else:
                                dst = dst_n[:, c * 30:c * 30 + 30]
                                stop_k = last if stop_n else -1
                            for kc in kcs:
                                nc.tensor.matmul(
                                    dst,
                                    wts[kc][:, g * 128:(g + 1) * 128],
                                    mov[:, kc * 30:(kc + 1) * 30],
                                    start=False,
                                    stop=(kc == stop_k))

                def elementwise(lab, s, hf, Trz, Tn, hfp, ginA, hf_new, hb_new):
                    """GRU combine for h-chunk half hf (cols hf*120..+120)."""
                    lo = hf * 120
                    t = lambda nm: ewp.tile(
                        [128, 120], F32, tag=f"{nm}{hf}{lab}",
                        name=f"{nm}_{lab}_{s}_{hf}")
                    rz = ewp.tile([128, 2, 120], F32, tag=f"rz{hf}{lab}",
                                  name=f"rz_{lab}_{s}_{hf}")
                    nc.scalar.activation(
                        rz,
                        Trz.rearrange("p (t x) -> p t x", t=2)[:, :, lo:lo + 120],
                        AF.Sigmoid)
                    oz = t("oz")
                    nc.vector.tensor_scalar(oz, rz[:, 1], -1.0, 1.0,
                                            OP.mult, OP.add)
                    bz = t("bz")
                    nc.vector.tensor_mul(bz, rz[:, 1], hfp[:, lo:lo + 120])
                    t1 = t("t1")
                    nc.vector.tensor_mul(t1, rz[:, 0], Tn[:, lo:lo + 120])
                    npre = t("np")
                    nc.vector.tensor_add(npre, t1, ginA)
                    nn = t("nn")
                    nc.scalar.activation(nn, npre, AF.Tanh)
                    aa = t("aa")
                    nc.vector.tensor_mul(aa, nn, oz)
                    nc.vector.tensor_add(hf_new[:, lo:lo + 120], aa, bz)
                    nc.vector.tensor_add(hb_new[:, lo:lo + 120], aa, bz)

                # Software pipeline: iteration i runs L1 matmuls of
                # step i and L2 matmuls of step i-1, so each elementwise
                # chain has a full matmul section of PE work to hide under.
                T1s, T2s = {}, {}
                h1 = {-1: (h1f, h1b)}
                h2 = {-1: (h2f, h2b)}

                for it in range(steps + 1):
                    sL1, sL2 = it, it - 1

                    if sL1 < steps:
                        b, j = divmod(sL1, 2)
                        T1 = psL.tile([128, 480], F32, tag="T1",
                                      name=f"T1_{sL1}")
                        T2 = psL.tile([128, 512], F32, tag="T2",
                                      name=f"T2_{sL1}")
                        T1s[sL1], T2s[sL1] = T1, T2
                        g = gtiles[b]
                        nc.tensor.matmul(T1, identr, g[:, j, 0:480],
                                         start=True, stop=False)
                        nc.tensor.matmul(T2[:, 0:256], identr, bhn1_sb,
                                         start=True, stop=False)
                        rec_mms(T1, T2[:, 0:240], w0, h1[sL1 - 1][1],
                                list(range(KC)), True, True)

                        # E1(sL1)
                        h1f_new = stp.tile([128, 240], F32, tag="h1f",
                                           name=f"h1f_{sL1}")
                        h1b_new = stp.tile([128, 240], BF16, tag="h1b",
                                           name=f"h1b_{sL1}")
                        for hf in range(2):
                            elementwise(
                                "a", sL1, hf, T1, T2[:, 0:240],
                                h1[sL1 - 1][0],
                                g[:, j, 480 + hf * 120:480 + hf * 120 + 120],
                                h1f_new, h1b_new)
                        h1[sL1] = (h1f_new, h1b_new)

                        if j == 0 and b + 2 < steps // 2:
                            gtiles.append(load_gi(b + 2))

                    if sL2 >= 0:
                        T3 = psL.tile([128, 480], F32, tag="T3",
                                      name=f"T3_{sL2}")
                        T4 = psL.tile([128, 256], F32, tag="T4",
                                      name=f"T4_{sL2}")
                        T2p = T2s.pop(sL2)
                        nc.tensor.matmul(T3, identr, b2rz_sb,
                                         start=True, stop=False)
                        nc.tensor.matmul(T2p[:, 256:512], identr, bhn2_sb,
                                         start=True, stop=False)
                        nc.tensor.matmul(T4, identr, bi1n_sb,
                                         start=True, stop=False)
                        # wi1 first (h1b(sL2) is a full iteration old), then
                        # wh1 (its h2b dep is the freshest elementwise)
                        rec_mms(T3, T4[:, 0:240], w1, h1[sL2][1],
                                list(range(KC)), False, True)
                        rec_mms(T3, T2p[:, 256:496], w2, h2[sL2 - 1][1],
                                list(range(KC)), True, True)

                        # E2(sL2)
                        h2f_new = stp.tile([128, 240], F32, tag="h2f",
                                           name=f"h2f_{sL2}")
                        h2b_new = stp.tile([128, 240], BF16, tag="h2b",
                                           name=f"h2b_{sL2}")
                        for hf in range(2):
                            elementwise(
                                "b", sL2, hf, T3, T2p[:, 256:496],
                                h2[sL2 - 1][0],
                                T4[:, hf * 120:hf * 120 + 120],
                                h2f_new, h2b_new)
                        h2[sL2] = (h2f_new, h2b_new)
                        nc.sync.dma_start(out=out[:, sL2, :], in_=h2f_new)

                        h1.pop(sL2 - 1, None)
                        h2.pop(sL2 - 2, None)
                        T1s.pop(sL2, None)

    nc.finalize()
    return nc


def ode_traj(w1, b1, w2, b2, w3, b3):
    """RK4 trajectory of the ODE, mirroring the reference exactly (fp32)."""
    w1t = w1.T.astype(np.float32)
    w2t = w2.T.astype(np.float32)
    w3t = w3.T.astype(np.float32)

    def f(h):
        a = np.tanh(h @ w1t + b1)
        a = np.tanh(a @ w2t + b2)
        return a @ w3t + b3

    dt = np.float32((1.0 / NSEG) / SUB)
    h = np.zeros((2, H), np.float32)
    traj = []
    for _ in range(NSEG):
        for _ in range(SUB):
            k1 = f(h)
            k2 = f(h + np.float32(0.5) * dt * k1)
            k3 = f(h + np.float32(0.5) * dt * k2)
            k4 = f(h + dt * k3)
            h = h + (dt / np.float32(6.0)) * (k1 + np.float32(2.0) * k2
                                              + np.float32(2.0) * k3 + k4)
        traj.append(h.copy())
    return np.stack(traj)  # (NSEG, 2, H)


def _bc_runs(per_gate, width):
    """[G] gate-vector -> [128, width] broadcast over 30 runs; G = n*128,
    cols laid out (chunk, run) with zero padding to `width`."""
    nchunk = per_gate.size // 128
    a = per_gate.reshape(nchunk, 128)  # [chunk, p]
    o = np.zeros((128, width), np.float32)
    o[:, :nchunk * 30] = np.repeat(
        a.T[:, :, None], 30, axis=2).reshape(128, nchunk * 30)
    return o


def make_in_maps(x, w1, b1, w2, b2, w3, b3, wi0, wh0, bi0, bh0,
                 wi1, wh1, bi1, bh1, cores=NCORES):
    traj = ode_traj(w1, b1, w2, b2, w3, b3)
    bf = ml_dtypes.bfloat16

    # xtr[p, kc, s, r] = x[s, r, kc*128+p]
    xtr = np.ascontiguousarray(
        x.reshape(S, T, KC, 128).transpose(3, 2, 0, 1)).astype(bf)

    bias1 = np.concatenate([bi0[:2 * H] + bh0[:2 * H], bi0[2 * H:]])

    shared = {
        "xtr": xtr,
        "wi0t": np.ascontiguousarray(
            wi0.T.reshape(KC, 128, 3 * H).transpose(1, 0, 2)).astype(bf),
        "wt0": np.ascontiguousarray(
            wh0.T.reshape(KC, 128, 3 * H).transpose(1, 0, 2)).astype(bf),
        "wt1": np.ascontiguousarray(
            wi1.T.reshape(KC, 128, 3 * H).transpose(1, 0, 2)).astype(bf),
        "wt2": np.ascontiguousarray(
            wh1.T.reshape(KC, 128, 3 * H).transpose(1, 0, 2)).astype(bf),
        "bias1t": np.ascontiguousarray(bias1.reshape(NG, 128).T),
        "b2rz": _bc_runs((bi1 + bh1)[:2 * H], 480),
        "bhn1": _bc_runs(bh0[2 * H:], 256),
        "bhn2": _bc_runs(bh1[2 * H:], 256),
        "bi1n": _bc_runs(bi1[2 * H:], 256),
    }
    in_maps = []
    for i in range(cores):
        m = dict(shared)
        for li, nm in ((0, "h1"), (1, "h2")):
            hf = np.repeat(traj[i, li].reshape(KC, 128).T[:, :, None],
                           30, axis=2).reshape(128, 240)
            m[f"{nm}f0"] = np.ascontiguousarray(hf)
            m[f"{nm}b0"] = np.ascontiguousarray(hf).astype(bf)
        in_maps.append(m)
    return in_maps


_NC_CACHE = {}


def _get_nc(steps=S):
    if steps not in _NC_CACHE:
        _NC_CACHE[steps] = build_nc(steps)
    return _NC_CACHE[steps]


def run_cores(inputs, steps=S, cores=NCORES, **run_kwargs):
    in_maps = make_in_maps(cores=cores, **inputs)
    nc = _get_nc(steps)
    return run_bass_kernel_spmd(nc, in_maps, core_ids=list(range(cores)),
                                **run_kwargs)


def kernel(x, w1, b1, w2, b2, w3, b3, wi0, wh0, bi0, bh0,
           wi1, wh1, bi1, bh1):
    args = dict(x=x, w1=w1, b1=b1, w2=w2, b2=b2, w3=w3, b3=b3,
                wi0=wi0, wh0=wh0, bi0=bi0, bh0=bh0,
                wi1=wi1, wh1=wh1, bi1=bi1, bh1=bh1)
    args = {k: np.asarray(v, np.float32) for k, v in args.items()}
    res = run_cores(args, steps=S, cores=NCORES)
    B = 64
    full = np.empty((B, T * NCORES, H), np.float32)
    for i in range(NCORES):
        o = np.asarray(res.results[i]["out"], np.float32)
        # out[p, s, c*30+t] -> full[s, t*8+i, c*128+p]
        full[:, i::NCORES, :] = o.reshape(
            128, S, KC, 30).transpose(1, 3, 2, 0).reshape(B, T, H)
    return full
